# revision 50
# baseline (speedup 1.0000x reference)
"""BioGNN message-passing kernel for 8 trn2 NeuronCores.

Strategy (sharding chosen per the "you choose" contract):
  - Shard by DESTINATION node range: core c owns nodes [c*125k, (c+1)*125k).
    Each edge is routed (host-side layout) to the core owning its dst, so no
    all-reduce is needed; the host concatenates per-core output slices.
  - Host does LAYOUT ONLY: per owned node, incoming edges are padded into
    dense ELL slabs binned by in-degree class; each slot carries a copy of
    x[src] in fp16. Node order inside a core is a host-known permutation
    (bin-major); outputs are un-permuted on the host.
  - Device does ALL arithmetic. v3 fast path (all-ones gains/node params):
    slabs are laid out as 128-slot COLUMNS (K edge slots x 128//K nodes per
    column); ScalarE/GpSimdE square the fp16 slabs in place; the TENSOR
    engine computes the per-node segment sums as block-diagonal ones-weight
    matmuls into PSUM (out = W.T @ x^2-slab, one 32-lane out block per
    matmul, stripes accumulate with start=False); VectorE runs only the
    elementwise tail per PSUM bank (1/(1+inh_sum), ratio, decay/growth).
    Degree classes {4,8,12,16,24,32,48,...} decompose into {16,8,4} passes
    that accumulate into the same PSUM element.
  - Mask-free tail: the no-act column prefix of asum is memset to 1.0 and
    matmuls OVERWRITE (start=True) columns of nodes that own act slots,
    reproducing `where(has_act, act_sum, 1)`; the (no-act, no-inh) bin
    columns are memset to 0 so `agg` lands at 0 there. Class promotion of
    rare bins never crosses the zero boundary, keeping both tricks exact.
  - v2 fallback path (general gains/node params) keeps the DVE
    square/multiply/tensor_reduce pipeline with bf16 slabs.
"""

import contextlib

import ml_dtypes
import numpy as np

import concourse.bacc as bacc
import concourse.mybir as mybir
import concourse.tile as tile
from concourse.bass_utils import run_bass_kernel_spmd

N_NODES = 1_000_000
N_CORES = 8
NPC = N_NODES // N_CORES
P = 128

F32 = mybir.dt.float32
BF16 = mybir.dt.bfloat16
F16 = mybir.dt.float16

# ---------------- v3 (tensor-engine) parameters ----------------
WINDOW = 3072        # slab window width per partition in f32 words
WC = WINDOW * 2      # fp16 slab columns per window
RARE_THRESH = 16384  # global node count below which a (ca, ci) pair is promoted
KCAP = 16
MAX_MM_N = 512       # moving-operand free-dim cap
SQ_SPLIT = 0.72      # fraction of window squares on ScalarE (rest GpSimdE)

# class value -> decomposition into base parts (descending)
def _cls_parts(v):
    parts = []
    while v >= 16:
        parts.append(16)
        v -= 16
    if v >= 8:
        parts.append(8)
        v -= 8
    if v >= 4:
        parts.append(4)
        v -= 4
    assert v == 0
    return parts


CLS_VALUES = [4, 8, 12, 16, 24, 32, 48, 64, 96, 128]
# W variant table: (K, q) -> column offset (in 32-col units) inside wmat
W_VARIANTS = [(4, 0), (8, 0), (8, 1), (16, 0), (16, 1), (16, 2), (16, 3)]
W_OFF = {kq: 32 * i for i, kq in enumerate(W_VARIANTS)}
W_COLS = 32 * len(W_VARIANTS)


def _class_of_v3(deg):
    bounds = np.array(CLS_VALUES)
    idx = np.searchsorted(bounds, deg, side="left")
    out = np.zeros_like(deg)
    nz = deg > 0
    out[nz] = bounds[np.minimum(idx[nz], len(bounds) - 1)]
    return out


def _build_wmat():
    w = np.zeros((P, W_COLS), np.float32)
    for (K, q), off in W_OFF.items():
        npcol = P // K
        p = np.arange(P)
        m = q * npcol + p // K
        w[p, off + m] = 1.0
    return w


def _pack_f16_words(arr):
    """[P, n] f32 -> [P, ceil(n/2)] f32 words holding round-to-nearest fp16."""
    a = arr.astype(np.float16)
    if a.shape[1] % 2:
        a = np.concatenate([a, np.zeros((a.shape[0], 1), np.float16)], axis=1)
    u = a.view(np.uint16)
    w = (u[:, 0::2].astype(np.uint32) | (u[:, 1::2].astype(np.uint32) << 16)).view(
        np.float32
    )
    return np.ascontiguousarray(w)


def _pack_bf16_words(arr):
    a = arr.astype(ml_dtypes.bfloat16)
    if a.shape[1] % 2:
        a = np.concatenate([a, np.zeros((a.shape[0], 1), ml_dtypes.bfloat16)], axis=1)
    u = a.view(np.uint16)
    w = (u[:, 0::2].astype(np.uint32) | (u[:, 1::2].astype(np.uint32) << 16)).view(
        np.float32
    )
    return np.ascontiguousarray(w)


def _make_bins(a_deg, i_deg):
    ca = _class_of_v3(a_deg)
    ci = _class_of_v3(i_deg)
    pair_id = ca * 1024 + ci
    uniq_p, cnt_p = np.unique(pair_id, return_counts=True)
    rare_pairs = set(uniq_p[cnt_p < RARE_THRESH].tolist())
    if rare_pairs:
        rare = np.isin(pair_id, list(rare_pairs))
        ca = np.where(rare & (ca > 0), np.maximum(ca, KCAP), ca)
        ci = np.where(rare & (ci > 0), np.maximum(ci, KCAP), ci)

    core_bins = []
    for c in range(N_CORES):
        lo, hi = c * NPC, (c + 1) * NPC
        nodes = np.arange(lo, hi)
        binid = ca[lo:hi] * 1024 + ci[lo:hi]
        order = np.argsort(binid, kind="stable")
        nodes_sorted = nodes[order]
        binid_sorted = binid[order]
        uniq, starts = np.unique(binid_sorted, return_index=True)
        ends = np.append(starts[1:], len(binid_sorted))
        bins = {}
        for u, s, e in zip(uniq, starts, ends):
            bins[(int(u) // 1024, int(u) % 1024)] = nodes_sorted[s:e]
        core_bins.append(bins)
    return core_bins


def _plan_v3(all_keys, nrows):
    """Emit the column stream + matmul entries (shared across cores).

    Column stream order: per Ca-group (same Ca, keys sorted):
      act parts (r, q, b) x group cols, then per Ci-run inh parts.
    Each mm entry: (voff, tab, o0, n, block, wcol0, start)
      voff: W column offset; tab: 'a'|'i'; o0: psum col; n: #cols;
      block: 32-lane block index; wcol0: slab col; start: psum overwrite.
    Entries are split at MAX_MM_N / psum-bank (512) / window (WC) boundaries.
    """
    row_off = {}
    off = 0
    for key in all_keys:
        row_off[key] = off
        off += nrows[key]
    NR = off

    mms = []
    sc = 0  # slab column cursor
    GUARD = 16  # unused cols at each window end (no rhs touches the tile edge)

    def emit(parts, tab, g0, g1):
        nonlocal sc
        off_r = 0
        for r, K in enumerate(parts):
            npcol = P // K
            for q in range(K // 4):
                for b in range(4):
                    start = (r == 0) and (q == 0)
                    c = g0
                    while c < g1:
                        if sc % 2:
                            sc += 1  # keep rhs word-aligned
                        n = min(g1 - c, MAX_MM_N, 512 - (c % 512))
                        # never cut a piece at the window boundary: if it
                        # doesn't fit in this window's usable space, move the
                        # whole piece to the next window
                        if (sc % WC) + n > WC - GUARD:
                            sc = (sc // WC + 1) * WC
                        mms.append(
                            (W_OFF[(K, q)], tab, c, n, b, sc, start, K, q, off_r)
                        )
                        sc += n
                        c += n
            off_r += K

    # groups of same Ca
    i = 0
    act_start = None
    while i < len(all_keys):
        j = i
        Ca = all_keys[i][0]
        while j < len(all_keys) and all_keys[j][0] == Ca:
            j += 1
        g0 = row_off[all_keys[i]]
        g1 = row_off[all_keys[j - 1]] + nrows[all_keys[j - 1]]
        if Ca > 0:
            if act_start is None:
                act_start = g0
            emit(_cls_parts(Ca), "a", g0, g1)
        # inh runs within the group
        ii = i
        while ii < j:
            jj = ii
            Ci = all_keys[ii][1]
            while jj < j and all_keys[jj][1] == Ci:
                jj += 1
            if Ci > 0:
                h0 = row_off[all_keys[ii]]
                h1 = row_off[all_keys[jj - 1]] + nrows[all_keys[jj - 1]]
                emit(_cls_parts(Ci), "i", h0, h1)
            ii = jj
        i = j
    if act_start is None:
        act_start = NR
    total_cols = sc
    n_windows = -(-total_cols // WC)
    return mms, total_cols, n_windows, row_off, NR, act_start


def _pack_v3(x, nu_ones, a_src, a_deg, a_ptr, i_src, i_deg, i_ptr, core_bins):
    all_keys = sorted({k for b in core_bins for k in b.keys()})
    nrows = {}
    for key in all_keys:
        nmax = max(len(b.get(key, ())) for b in core_bins)
        nrows[key] = -(-nmax // P)

    mms, total_cols, n_windows, row_off, NR, act_start = _plan_v3(all_keys, nrows)

    # per-core slab construction: iterate the same emission order
    wmat = _build_wmat()
    per_core = []
    meta_orders = []
    for c in range(N_CORES):
        bins = core_bins[c]
        # L grid per bin: flat nr*P, node = L[lane*nr + col]
        Ls = {}
        orders = []
        for key in all_keys:
            nr = nrows[key]
            L = np.full(nr * P, -1, np.int64)
            have = bins.get(key)
            if have is not None:
                L[: len(have)] = have
            Ls[key] = L.reshape(P, nr)  # [lane, col]
            orders.append((key, L))
        meta_orders.append(orders)

        # node grid over global cols: node_at[lane, gcol]
        node_grid = np.concatenate([Ls[key] for key in all_keys], axis=1)
        assert node_grid.shape == (P, NR)

        slab_cols = np.zeros((P, total_cols + (total_cols % 2)), np.float32)
        pp = np.arange(P)
        for voff, tab, o0, n, b, wcol0, start, K, q, off_r in mms:
            src, deg, ptr = (a_src, a_deg, a_ptr) if tab == "a" else (
                i_src, i_deg, i_ptr)
            npcol = P // K
            jj = pp // K
            kk = pp % K
            lanes = 32 * b + q * npcol + jj  # [P]
            nodes = node_grid[lanes, o0 : o0 + n]  # [P, n]
            nd = np.where(nodes >= 0, nodes, 0)
            d = np.where(nodes >= 0, deg[nd], 0)
            eidx = off_r + kk[:, None]
            valid = eidx < d
            gidx = ptr[nd] + np.where(valid, eidx, 0)
            vals = np.where(valid, x[src[gidx]], np.float32(0))
            slab_cols[:, wcol0 : wcol0 + n] = vals

        nvf = np.zeros((P, NR), np.float32)
        valid = node_grid >= 0
        nvf[valid] = x[node_grid[valid]]

        per_core.append(
            {
                "slab": _pack_f16_words(slab_cols[:, : total_cols + (total_cols % 2)]),
                "nodevf": np.ascontiguousarray(nvf),
                "wmat": _pack_f16_words(wmat),
            }
        )

    r00 = None
    if (0, 0) in nrows:
        r00 = (row_off[(0, 0)], nrows[(0, 0)])

    i0_ranges = []
    for key in all_keys:
        if key[1] == 0:
            a, bnd = row_off[key], row_off[key] + nrows[key]
            if i0_ranges and i0_ranges[-1][1] == a:
                i0_ranges[-1] = (i0_ranges[-1][0], bnd)
            else:
                i0_ranges.append((a, bnd))

    shapes = {
        "v3": True,
        "i0_ranges": i0_ranges,
        "keys": all_keys,
        "nrows": nrows,
        "NR": NR,
        "mms": mms,
        "total_cols": total_cols,
        "n_windows": n_windows,
        "act_start": act_start,
        "r00": r00,
    }
    return per_core, meta_orders, shapes


def _build_nc_v3(shapes, loop_R=None, variant="full", debug=False):
    NR = shapes["NR"]
    mms = shapes["mms"]
    n_windows = shapes["n_windows"]
    total_cols = shapes["total_cols"]
    act_start = shapes["act_start"]
    r00 = shapes["r00"]

    n_banks = -(-NR // 512)
    bank_cols = [min(512, NR - 512 * k) for k in range(n_banks)]

    slab_words = (total_cols + 1) // 2
    wmat_words = W_COLS // 2

    nc = bacc.Bacc(None, target_bir_lowering=False)
    sl_d = nc.declare_dram_parameter("slab", [P, slab_words], F32, isOutput=False)
    nvf_d = nc.declare_dram_parameter("nodevf", [P, NR], F32, isOutput=False)
    wm_d = nc.declare_dram_parameter("wmat", [P, wmat_words], F32, isOutput=False)
    out_d = nc.declare_dram_parameter("out", [P, NR], F32, isOutput=True)
    if debug:
        dsq_d = nc.declare_dram_parameter(
            "dbg_sq", [P, slab_words], F32, isOutput=True
        )
        dpa_d = nc.declare_dram_parameter(
            "dbg_pa", [P, 512 * n_banks], F32, isOutput=True
        )
        dpi_d = nc.declare_dram_parameter(
            "dbg_pi", [P, 512 * n_banks], F32, isOutput=True
        )

    MUL = mybir.AluOpType.mult
    ADD = mybir.AluOpType.add

    do_windows = variant not in ("noop", "empty")
    do_sq = variant in ("full", "sq", "notail")
    do_mm = variant in ("full", "notail")
    do_tail = variant == "full"

    with tile.TileContext(nc) as tc:
        with (
            tc.tile_pool(name="slab", bufs=4) as slab_tp,
            tc.tile_pool(name="node", bufs=1) as node_tp,
            tc.tile_pool(name="wm", bufs=1) as wm_tp,
            tc.tile_pool(name="tail", bufs=1) as tail_tp,
            tc.psum_pool(name="ps", bufs=1) as ps_tp,
        ):
            loop_cm = tc.For_i(0, loop_R, 1) if loop_R else contextlib.nullcontext()
            with loop_cm:
                pa = [
                    ps_tp.tile([P, 512], F32, tag=f"pa{k}", name=f"pa{k}")
                    for k in range(n_banks)
                ]
                pi = [
                    ps_tp.tile([P, 512], F32, tag=f"pi{k}", name=f"pi{k}")
                    for k in range(n_banks)
                ]
                pdump = ps_tp.tile([P, 512], F32, tag="pdump", name="pdump")
                ptiles = {"a": pa, "i": pi}

                if variant == "noop":
                    nc.vector.memset(pa[0][:, :1], 0.0)
                    act_start_eff = 0
                else:
                    act_start_eff = act_start
                # init: no-act prefix of asum = 1.0 (num = 1 when no
                # activators); (0,0) bin = 0.0 (agg = 0 when no edges)
                c0 = 0
                while c0 < act_start_eff:
                    k = c0 // 512
                    n = min(act_start_eff - c0, 512 * (k + 1) - c0)
                    nc.vector.memset(pa[k][:, c0 - 512 * k : c0 - 512 * k + n], 1.0)
                    c0 += n
                if r00 is not None and variant != "noop":
                    k = r00[0] // 512
                    assert (r00[0] + r00[1] - 1) // 512 == k
                    nc.vector.memset(
                        pa[k][:, r00[0] - 512 * k : r00[0] - 512 * k + r00[1]], 0.0
                    )
                # isum = 0 only where no inh matmul writes: the (Ka, 0) bins
                # (all other columns are fully overwritten by q0/start=True)
                if variant != "noop":
                    for z0, z1 in shapes["i0_ranges"]:
                        c0 = z0
                        while c0 < z1:
                            k = c0 // 512
                            n = min(z1 - c0, 512 * (k + 1) - c0)
                            nc.vector.memset(
                                pi[k][:, c0 - 512 * k : c0 - 512 * k + n], 0.0
                            )
                            c0 += n

                wm = wm_tp.tile([P, wmat_words], F32, tag="wm")
                wm_f16 = wm[:, :].bitcast(F16)
                nvf = node_tp.tile([P, NR], F32, tag="nvf")
                if variant != "noop":
                    nc.sync.dma_start(out=wm[:, :], in_=wm_d[:, :])
                    nc.scalar.dma_start(out=nvf[:, :], in_=nvf_d[:, :])

                by_win = {}
                for e in mms:
                    by_win.setdefault(e[5] // WC, []).append(e)

                wtiles = []
                for win in range(n_windows) if do_windows else ():
                    wt = slab_tp.tile([P, WINDOW], F32, tag="win")
                    wtiles.append(wt)
                    w0 = win * WINDOW
                    used = min(WINDOW, slab_words - w0)
                    dma_eng = nc.sync if win % 2 == 0 else nc.scalar
                    half = (used // 2 + 63) // 64 * 64
                    half = min(half, used)
                    dma_eng.dma_start(out=wt[:, :half], in_=sl_d[:, w0 : w0 + half])
                    if half < used:
                        dma_eng.dma_start(
                            out=wt[:, half:used], in_=sl_d[:, w0 + half : w0 + used]
                        )
                    # square in place: ScalarE takes the first chunk, GpSimdE
                    # the rest
                    wf = wt[:, :used].bitcast(F16)
                    ncols = used * 2
                    s_cols = int(ncols * SQ_SPLIT) // 2 * 2
                    if do_sq and s_cols > 0:
                        xs = wf[:, :s_cols]
                        nc.scalar.square(out=xs, in_=xs)
                    if do_sq and s_cols < ncols:
                        xs = wf[:, s_cols:ncols]
                        nc.gpsimd.tensor_tensor(out=xs, in0=xs, in1=xs, op=MUL)

                    if debug:
                        nc.sync.dma_start(
                            out=dsq_d[:, w0 : w0 + used], in_=wt[:, :used]
                        )
                    for voff, tab, o0, n, b, wcol0, start, _K, _q, _ofr in (
                        by_win.get(win, ()) if do_mm else ()
                    ):
                        k = o0 // 512
                        oo = o0 - 512 * k
                        wc = wcol0 - win * WC
                        nc.tensor.matmul(
                            out=ptiles[tab][k][32 * b : 32 * b + 32, oo : oo + n],
                            lhsT=wm_f16[:, voff : voff + 32],
                            rhs=wf[:, wc : wc + n],
                            start=start,
                            stop=True,
                            skip_group_check=True,
                            tile_position=(0, 32 * b),
                        )
                    # sacrificial trailing matmul: the scheduler's window-end
                    # event boundary voids the last matmul's PSUM write on HW;
                    # park it on a dump bank so every real write survives.
                    nc.tensor.matmul(
                        out=pdump[0:32, 0:16],
                        lhsT=wm_f16[:, 0:32],
                        rhs=wf[:, 0:16],
                        start=True,
                        stop=True,
                        skip_group_check=True,
                        tile_position=(0, 0),
                    )

                if debug:
                    dcp = tail_tp.tile([P, 512], F32, tag="dcp")
                    for k in range(n_banks):
                        nc.vector.tensor_copy(out=dcp[:, :], in_=pa[k][:, :])
                        nc.sync.dma_start(
                            out=dpa_d[:, 512 * k : 512 * (k + 1)], in_=dcp[:, :]
                        )
                        nc.vector.tensor_copy(out=dcp[:, :], in_=pi[k][:, :])
                        nc.sync.dma_start(
                            out=dpi_d[:, 512 * k : 512 * (k + 1)], in_=dcp[:, :]
                        )

                # elementwise tail, per psum bank
                ot = tail_tp.tile([P, NR], F32, tag="ot")
                den = tail_tp.tile([P, 512], F32, tag="den")
                rde = tail_tp.tile([P, 512], F32, tag="rde")
                scr = tail_tp.tile([P, 512], F32, tag="scr")
                if not do_tail and variant != "noop":
                    nc.sync.dma_start(out=out_d[:, :], in_=nvf[:, :])
                for k in range(n_banks) if do_tail else ():
                    n = bank_cols[k]
                    dn = den[:, :n]
                    rd = rde[:, :n]
                    sc_ = scr[:, :n]
                    o = ot[:, 512 * k : 512 * k + n]
                    nc.vector.tensor_scalar_add(dn, pi[k][:, :n], 1.0)
                    nc.vector.reciprocal_approx_accurate(out=rd, in_=dn, scratch=sc_)
                    nc.vector.tensor_tensor(out=rd, in0=pa[k][:, :n], in1=rd, op=MUL)
                    # out = agg - x + 1
                    nc.vector.scalar_tensor_tensor(
                        out=o, in0=nvf[:, 512 * k : 512 * k + n], scalar=-1.0,
                        in1=rd, op0=MUL, op1=ADD,
                    )
                    nc.vector.tensor_scalar_add(o, o, 1.0)
                    dma_eng = nc.sync if k % 2 == 0 else nc.scalar
                    dma_eng.dma_start(
                        out=out_d[:, 512 * k : 512 * k + n], in_=o
                    )

    nc.finalize()
    return nc


# ================= v2 (DVE) fallback path =================
CHUNK_SLOTS = 4096
V2_WINDOW = 3072


def _degree_classes(max_deg):
    ks = [4, 6, 8, 12, 16, 32]
    while ks[-1] < max_deg:
        ks.append(ks[-1] * 2)
    return ks


def _class_of(deg, ks):
    bounds = np.array(ks)
    idx = np.searchsorted(bounds, deg, side="left")
    out = np.zeros_like(deg)
    nz = deg > 0
    out[nz] = bounds[idx[nz]]
    return out


def _make_plan_v2(all_keys, nrows, fast_k):
    row_off = {}
    off = 0
    for key in all_keys:
        row_off[key] = off
        off += nrows[key]
    total_rows = off

    chunks = []
    act_classes = []
    for key in all_keys:
        if key[0] > 0 and (not act_classes or act_classes[-1][0] != key[0]):
            act_classes.append((key[0], row_off[key]))
    act_seg_rows = {}
    for Ka, seg0 in act_classes:
        seg_rows = sum(nrows[k] for k in all_keys if k[0] == Ka)
        act_seg_rows[Ka] = (seg0, seg_rows)
        T = max(1, CHUNK_SLOTS // Ka)
        r0 = 0
        while r0 < seg_rows:
            t = min(T, seg_rows - r0)
            chunks.append(("a", Ka, seg0 + r0, t))
            r0 += t
    for key in all_keys:
        Ki = key[1]
        if Ki == 0:
            continue
        nr = nrows[key]
        T = max(1, CHUNK_SLOTS // Ki)
        r0 = 0
        while r0 < nr:
            t = min(T, nr - r0)
            chunks.append(("i", Ki, row_off[key] + r0, t))
            r0 += t

    entries = []
    wins = []
    for table, K, g0, t in sorted(chunks, key=lambda c: -(c[3] * c[1])):
        w = t * K
        assert w % 2 == 0
        cw = w // 2 if fast_k else w
        for wi in range(len(wins)):
            if wins[wi] >= cw:
                break
        else:
            wins.append(V2_WINDOW)
            wi = len(wins) - 1
        woff = V2_WINDOW - wins[wi]
        entries.append((table, K, g0, t, wi, woff))
        wins[wi] -= cw
    n_windows = len(wins)
    win_used = [-(-(V2_WINDOW - rem) // 64) * 64 for rem in wins]
    win_start = [0]
    for u in win_used[:-1]:
        win_start.append(win_start[-1] + u)
    return entries, n_windows, win_used, win_start, row_off, act_seg_rows, total_rows


def _pack_v2(x, k_act, k_inh, nu, decay, growth, act_src, act_dst, inh_src, inh_dst,
             fast_k, fast_nodev, tables):
    a_src, a_k, a_deg, a_ptr, i_src, i_k, i_deg, i_ptr = tables

    max_deg = int(max(a_deg.max(), i_deg.max()))
    ks = _degree_classes(max_deg)
    nclasses = len(ks) + 1
    klist = [0] + ks

    ca = _class_of(a_deg, ks)
    ci = _class_of(i_deg, ks)

    kcap = min(16, ks[-1])
    pair_id = ca * 1024 + ci
    uniq_p, cnt_p = np.unique(pair_id, return_counts=True)
    rare_pairs = set(uniq_p[cnt_p < RARE_THRESH].tolist())
    if rare_pairs:
        rare = np.isin(pair_id, list(rare_pairs))
        ca = np.where(rare & (ca > 0), np.maximum(ca, kcap), ca)
        ci = np.where(rare & (ci > 0), np.maximum(ci, kcap), ci)

    core_bins = []
    for c in range(N_CORES):
        lo, hi = c * NPC, (c + 1) * NPC
        nodes = np.arange(lo, hi)
        binid = np.searchsorted(np.array(klist), ca[lo:hi]) * nclasses + np.searchsorted(
            np.array(klist), ci[lo:hi]
        )
        order = np.argsort(binid, kind="stable")
        nodes_sorted = nodes[order]
        binid_sorted = binid[order]
        uniq, starts = np.unique(binid_sorted, return_index=True)
        ends = np.append(starts[1:], len(binid_sorted))
        bins = {}
        for u, s, e in zip(uniq, starts, ends):
            bins[(klist[u // nclasses], klist[u % nclasses])] = nodes_sorted[s:e]
        core_bins.append(bins)

    all_keys = sorted({k for b in core_bins for k in b.keys()})
    nrows = {}
    for key in all_keys:
        nmax = max(len(b.get(key, ())) for b in core_bins)
        nrows[key] = -(-nmax // P)

    (entries, n_windows, win_used, win_start, row_off, act_seg_rows,
     total_rows) = _make_plan_v2(all_keys, nrows, fast_k)

    def build_slab(L, K, rowptr, deg, srcs, kvals, want_k):
        Lc = L.clip(0)
        d = np.where(L >= 0, deg[Lc], 0)
        base = rowptr[Lc]
        cols = np.arange(K)
        idx2 = base[:, None] + cols[None, :]
        valid = cols[None, :] < d[:, None]
        idxc = np.where(valid, idx2, 0)
        sx = np.where(valid, x[srcs[idxc]], np.float32(0)).astype(np.float32)
        sk = None
        if want_k:
            sk = np.where(valid, kvals[idxc], np.float32(0)).astype(np.float32)
        return sx, sk

    per_core = []
    meta_orders = []
    for c in range(N_CORES):
        bins = core_bins[c]
        ax_seg = {}
        ak_seg = {}
        ix_bin = {}
        ik_bin = {}
        xv_l = []
        nodev_l = {"nuv": [], "dev": [], "grv": []}
        orders = []
        ax_parts = {}
        ak_parts = {}
        for key in all_keys:
            Ka, Ki = key
            nr = nrows[key]
            L = np.full(nr * P, -1, np.int64)
            have = bins.get(key)
            if have is not None:
                L[: len(have)] = have
            orders.append((key, L))
            if Ka > 0:
                sx, sk = build_slab(L, Ka, a_ptr, a_deg, a_src, a_k, not fast_k)
                ax_parts.setdefault(Ka, []).append(sx.reshape(P, nr * Ka))
                if not fast_k:
                    ak_parts.setdefault(Ka, []).append(sk.reshape(P, nr * Ka))
            if Ki > 0:
                sx, sk = build_slab(L, Ki, i_ptr, i_deg, i_src, i_k, not fast_k)
                ix_bin[key] = sx.reshape(P, nr * Ki)
                if not fast_k:
                    ik_bin[key] = sk.reshape(P, nr * Ki)
            valid = L >= 0
            Lc = L.clip(0)

            def pk(v):
                return (
                    np.where(valid, v[Lc], np.float32(0))
                    .astype(np.float32)
                    .reshape(P, nr)
                )

            xv_l.append(pk(x))
            if not fast_nodev:
                nodev_l["nuv"].append(pk(nu))
                nodev_l["dev"].append(pk(decay))
                nodev_l["grv"].append(pk(growth))

        for Ka, parts in ax_parts.items():
            ax_seg[Ka] = np.concatenate(parts, axis=1)
            if not fast_k:
                ak_seg[Ka] = np.concatenate(ak_parts[Ka], axis=1)

        slab = np.zeros((P, win_start[-1] + win_used[-1]), np.float32)
        for table, K, g0, t, win, woff in entries:
            w = t * K
            base = win_start[win] + woff
            if table == "a":
                seg0, _ = act_seg_rows[K]
                r0 = g0 - seg0
                sx = ax_seg[K][:, r0 * K : (r0 + t) * K]
                sk = ak_seg[K][:, r0 * K : (r0 + t) * K] if not fast_k else None
            else:
                key = next(
                    kk for kk in all_keys
                    if kk[1] == K and row_off[kk] <= g0 < row_off[kk] + nrows[kk]
                )
                r0 = g0 - row_off[key]
                sx = ix_bin[key][:, r0 * K : (r0 + t) * K]
                sk = ik_bin[key][:, r0 * K : (r0 + t) * K] if not fast_k else None
            xw = w // 2
            slab[:, base : base + xw] = _pack_bf16_words(sx)
            if not fast_k:
                slab[:, base + xw : base + 2 * xw] = _pack_bf16_words(sk)

        core = {
            "slab": slab,
            "nodevf": np.ascontiguousarray(np.concatenate(xv_l, axis=1)),
        }
        if not fast_nodev:
            nodevb = np.concatenate(
                [np.concatenate(nodev_l[nm], axis=1) for nm in ("nuv", "dev", "grv")],
                axis=1,
            )
            core["nodevb"] = _pack_bf16_words(nodevb)
        per_core.append(core)
        meta_orders.append(orders)

    r00 = None
    if (0, 0) in nrows:
        r00 = (row_off[(0, 0)], nrows[(0, 0)])

    shapes = {
        "v3": False,
        "keys": all_keys,
        "nrows": nrows,
        "NR": total_rows,
        "entries": entries,
        "n_windows": n_windows,
        "win_used": win_used,
        "win_start": win_start,
        "fast_k": fast_k,
        "fast_nodev": fast_nodev,
        "r00": r00,
    }
    return per_core, meta_orders, shapes


def _build_nc_v2(shapes, loop_R=None):
    NR = shapes["NR"]
    entries = shapes["entries"]
    n_windows = shapes["n_windows"]
    win_used = shapes["win_used"]
    win_start = shapes["win_start"]
    fast_k = shapes["fast_k"]
    fast_nodev = shapes["fast_nodev"]
    r00 = shapes["r00"]

    NB = (3 * NR + 1) // 2
    nc = bacc.Bacc(None, target_bir_lowering=False)
    sl_d = nc.declare_dram_parameter(
        "slab", [P, win_start[-1] + win_used[-1]], F32, isOutput=False
    )
    nvf_d = nc.declare_dram_parameter("nodevf", [P, NR], F32, isOutput=False)
    nvb_d = None
    if not fast_nodev:
        nvb_d = nc.declare_dram_parameter("nodevb", [P, NB], F32, isOutput=False)
    out_d = nc.declare_dram_parameter("out", [P, NR], F32, isOutput=True)

    MUL = mybir.AluOpType.mult
    ADD = mybir.AluOpType.add
    X = mybir.AxisListType.X

    with tile.TileContext(nc) as tc:
        with (
            tc.tile_pool(name="slab", bufs=3) as slab_tp,
            tc.tile_pool(name="sums", bufs=1) as sums_tp,
            tc.tile_pool(name="node", bufs=1) as node_tp,
        ):
            loop_cm = tc.For_i(0, loop_R, 1) if loop_R else contextlib.nullcontext()
            with loop_cm:
                asum = sums_tp.tile([P, NR], F32, tag="asum")
                isum = sums_tp.tile([P, NR], F32, tag="isum")
                nc.vector.memset(asum[:, :], 1.0)
                nc.vector.memset(isum[:, :], 0.0)
                if r00 is not None:
                    nc.vector.memset(asum[:, r00[0] : r00[0] + r00[1]], 0.0)
                bufs = {"a": asum, "i": isum}

                nvf = node_tp.tile([P, NR], F32, tag="nvf")
                nc.scalar.dma_start(out=nvf[:, :], in_=nvf_d[:, :])
                if not fast_nodev:
                    nvb = node_tp.tile([P, NB], F32, tag="nvb")
                    nc.scalar.dma_start(out=nvb[:, :], in_=nvb_d[:, :])
                    nvb_b = nvb[:, :].bitcast(BF16)
                    iv = {}
                    for j, nm in enumerate(("nuv", "dev", "grv")):
                        iv[nm] = nvb_b[:, j * NR : (j + 1) * NR]

                by_win = {}
                for e in entries:
                    by_win.setdefault(e[4], []).append(e)
                sq_engine = 0
                for win in range(n_windows):
                    wt = slab_tp.tile([P, V2_WINDOW], F32, tag="win")
                    used = win_used[win]
                    half = (used // 2 + 63) // 64 * 64
                    half = min(half, used)
                    dma_eng = nc.sync if win % 2 == 0 else nc.scalar
                    dma_eng.dma_start(
                        out=wt[:, :half],
                        in_=sl_d[:, win_start[win] : win_start[win] + half],
                    )
                    if half < used:
                        dma_eng.dma_start(
                            out=wt[:, half:used],
                            in_=sl_d[:, win_start[win] + half : win_start[win] + used],
                        )
                    for table, K, g0, t, _win, woff in by_win.get(win, ()):
                        w = t * K
                        xw = w // 2
                        xs = wt[:, woff : woff + xw].bitcast(BF16)
                        if fast_k:
                            if sq_engine == 0:
                                nc.scalar.square(out=xs, in_=xs)
                            else:
                                nc.gpsimd.tensor_tensor(out=xs, in0=xs, in1=xs, op=MUL)
                            sq_engine ^= 1
                        else:
                            kS = wt[:, woff + xw : woff + 2 * xw].bitcast(BF16)
                            nc.scalar.square(out=xs, in_=xs)
                            nc.gpsimd.tensor_tensor(out=xs, in0=xs, in1=kS, op=MUL)
                        nc.vector.tensor_reduce(
                            out=bufs[table][:, g0 : g0 + t],
                            in_=xs.rearrange("p (t k) -> p t k", k=K),
                            axis=X,
                            op=ADD,
                        )

                den = node_tp.tile([P, NR], F32, tag="den")
                rde = node_tp.tile([P, NR], F32, tag="rde")
                scr = node_tp.tile([P, NR], F32, tag="scr")
                ot = node_tp.tile([P, NR], F32, tag="ot")
                A = lambda tl: tl[:, :]

                nc.vector.tensor_scalar_add(A(den), A(isum), 1.0)
                nc.vector.reciprocal_approx_accurate(
                    out=A(rde), in_=A(den), scratch=A(scr)
                )
                nc.vector.tensor_tensor(out=A(rde), in0=A(asum), in1=A(rde), op=MUL)
                if fast_nodev:
                    nc.vector.scalar_tensor_tensor(
                        out=A(ot), in0=A(nvf), scalar=-1.0, in1=A(rde),
                        op0=MUL, op1=ADD,
                    )
                    nc.vector.tensor_scalar_add(A(ot), A(ot), 1.0)
                else:
                    nc.vector.tensor_tensor(out=A(ot), in0=iv["nuv"], in1=A(rde), op=MUL)
                    nc.vector.scalar_tensor_tensor(
                        out=A(scr), in0=iv["dev"], scalar=-1.0, in1=A(nvf),
                        op0=MUL, op1=MUL,
                    )
                    nc.vector.tensor_tensor(out=A(ot), in0=A(ot), in1=A(scr), op=ADD)
                    nc.vector.tensor_tensor(out=A(ot), in0=A(ot), in1=iv["grv"], op=ADD)
                nc.scalar.dma_start(out=out_d[:, :], in_=ot[:, :])

    nc.finalize()
    return nc


# ================= dispatchers =================
def _pack(x, k_act, k_inh, nu, decay, growth, act_src, act_dst, inh_src, inh_dst):
    fast_k = bool(np.all(k_act == 1.0) and np.all(k_inh == 1.0))
    fast_nodev = bool(
        np.all(nu == 1.0) and np.all(decay == 1.0) and np.all(growth == 1.0)
    )

    def sorted_table(src, dst, k):
        order = np.argsort(dst, kind="stable")
        deg = np.bincount(dst, minlength=N_NODES).astype(np.int64)
        rowptr = np.zeros(N_NODES + 1, np.int64)
        np.cumsum(deg, out=rowptr[1:])
        return src[order], k[order], deg, rowptr

    a_src, a_k, a_deg, a_ptr = sorted_table(act_src, act_dst, k_act)
    i_src, i_k, i_deg, i_ptr = sorted_table(inh_src, inh_dst, k_inh)

    if fast_k and fast_nodev:
        core_bins = _make_bins(a_deg, i_deg)
        return _pack_v3(
            x, None, a_src, a_deg, a_ptr, i_src, i_deg, i_ptr, core_bins
        )
    return _pack_v2(
        x, k_act, k_inh, nu, decay, growth, act_src, act_dst, inh_src, inh_dst,
        fast_k, fast_nodev,
        (a_src, a_k, a_deg, a_ptr, i_src, i_k, i_deg, i_ptr),
    )


def _build_nc(shapes, loop_R=None, variant="full"):
    if shapes.get("v3"):
        return _build_nc_v3(shapes, loop_R=loop_R, variant=variant)
    return _build_nc_v2(shapes, loop_R=loop_R)


def kernel(**inputs) -> np.ndarray:
    per_core, meta_orders, shapes = _pack(
        np.asarray(inputs["x"], np.float32),
        np.asarray(inputs["k_act"], np.float32),
        np.asarray(inputs["k_inh"], np.float32),
        np.asarray(inputs["nu"], np.float32),
        np.asarray(inputs["decay"], np.float32),
        np.asarray(inputs["growth"], np.float32),
        np.asarray(inputs["act_src"]),
        np.asarray(inputs["act_dst"]),
        np.asarray(inputs["inh_src"]),
        np.asarray(inputs["inh_dst"]),
    )
    nc = _build_nc(shapes)
    in_maps = [dict(per_core[c]) for c in range(N_CORES)]
    res = run_bass_kernel_spmd(nc, in_maps, list(range(N_CORES)))

    out_full = np.zeros(N_NODES, np.float32)
    nrows = shapes["nrows"]
    for c in range(N_CORES):
        arr = res.results[c]["out"]
        offN = 0
        for key, L in meta_orders[c]:
            nr = nrows[key]
            block = arr[:, offN : offN + nr].reshape(P * nr)
            valid = L >= 0
            out_full[L[valid]] = block[valid]
            offN += nr
    return out_full


# revision 55
# speedup vs baseline: 1.1667x; 1.1667x over previous
"""BioGNN message-passing kernel for 8 trn2 NeuronCores.

Strategy (sharding chosen per the "you choose" contract):
  - Shard by DESTINATION node range: core c owns nodes [c*125k, (c+1)*125k).
    Each edge is routed (host-side layout) to the core owning its dst, so no
    all-reduce is needed; the host concatenates per-core output slices.
  - Host does LAYOUT ONLY: per owned node, incoming edges are padded into
    dense ELL slabs binned by in-degree class; each slot carries a copy of
    x[src] in fp16. Node order inside a core is a host-known permutation
    (bin-major); outputs are un-permuted on the host.
  - Device does ALL arithmetic. v3 fast path (all-ones gains/node params):
    slabs are laid out as 128-slot COLUMNS (K edge slots x 128//K nodes per
    column); ScalarE/GpSimdE square the fp16 slabs in place; the TENSOR
    engine computes the per-node segment sums as block-diagonal ones-weight
    matmuls into PSUM (out = W.T @ x^2-slab, one 32-lane out block per
    matmul, stripes accumulate with start=False); VectorE runs only the
    elementwise tail per PSUM bank (1/(1+inh_sum), ratio, decay/growth).
    Degree classes {4,8,12,16,24,32,48,...} decompose into {16,8,4} passes
    that accumulate into the same PSUM element.
  - Mask-free tail: the no-act column prefix of asum is memset to 1.0 and
    matmuls OVERWRITE (start=True) columns of nodes that own act slots,
    reproducing `where(has_act, act_sum, 1)`; the (no-act, no-inh) bin
    columns are memset to 0 so `agg` lands at 0 there. Class promotion of
    rare bins never crosses the zero boundary, keeping both tricks exact.
  - v2 fallback path (general gains/node params) keeps the DVE
    square/multiply/tensor_reduce pipeline with bf16 slabs.
"""

import contextlib

import ml_dtypes
import numpy as np

import concourse.bacc as bacc
import concourse.mybir as mybir
import concourse.tile as tile
from concourse.bass_utils import run_bass_kernel_spmd

N_NODES = 1_000_000
N_CORES = 8
NPC = N_NODES // N_CORES
P = 128

F32 = mybir.dt.float32
BF16 = mybir.dt.bfloat16
F16 = mybir.dt.float16

# ---------------- v3 (tensor-engine) parameters ----------------
WINDOW = 3072        # slab window width per partition in f32 words
WC = WINDOW * 2      # fp16 slab columns per window
RARE_THRESH = 16384  # global node count below which a (ca, ci) pair is promoted
KCAP = 16
MAX_MM_N = 512       # moving-operand free-dim cap
SQ_SPLIT = 0.72      # fraction of window squares on ScalarE (rest GpSimdE)

# class value -> decomposition into base parts (descending)
def _cls_parts(v):
    parts = []
    while v >= 16:
        parts.append(16)
        v -= 16
    if v >= 8:
        parts.append(8)
        v -= 8
    if v >= 4:
        parts.append(4)
        v -= 4
    assert v == 0
    return parts


CLS_VALUES = [4, 8, 12, 16, 24, 32, 48, 64, 96, 128]
# W variant table: (K, q) -> column offset (in 32-col units) inside wmat
W_VARIANTS = [(4, 0), (8, 0), (8, 1), (16, 0), (16, 1), (16, 2), (16, 3)]
W_OFF = {kq: 32 * i for i, kq in enumerate(W_VARIANTS)}
W_COLS = 32 * len(W_VARIANTS)


def _class_of_v3(deg):
    bounds = np.array(CLS_VALUES)
    idx = np.searchsorted(bounds, deg, side="left")
    out = np.zeros_like(deg)
    nz = deg > 0
    out[nz] = bounds[np.minimum(idx[nz], len(bounds) - 1)]
    return out


def _build_wmat():
    w = np.zeros((P, W_COLS), np.float32)
    for (K, q), off in W_OFF.items():
        npcol = P // K
        p = np.arange(P)
        m = q * npcol + p // K
        w[p, off + m] = 1.0
    return w


def _pack_f16_words(arr):
    """[P, n] f32 -> [P, ceil(n/2)] f32 words holding round-to-nearest fp16."""
    a = arr.astype(np.float16)
    if a.shape[1] % 2:
        a = np.concatenate([a, np.zeros((a.shape[0], 1), np.float16)], axis=1)
    u = a.view(np.uint16)
    w = (u[:, 0::2].astype(np.uint32) | (u[:, 1::2].astype(np.uint32) << 16)).view(
        np.float32
    )
    return np.ascontiguousarray(w)


def _pack_bf16_words(arr):
    a = arr.astype(ml_dtypes.bfloat16)
    if a.shape[1] % 2:
        a = np.concatenate([a, np.zeros((a.shape[0], 1), ml_dtypes.bfloat16)], axis=1)
    u = a.view(np.uint16)
    w = (u[:, 0::2].astype(np.uint32) | (u[:, 1::2].astype(np.uint32) << 16)).view(
        np.float32
    )
    return np.ascontiguousarray(w)


def _make_bins(a_deg, i_deg):
    ca = _class_of_v3(a_deg)
    ci = _class_of_v3(i_deg)
    pair_id = ca * 1024 + ci
    uniq_p, cnt_p = np.unique(pair_id, return_counts=True)
    rare_pairs = set(uniq_p[cnt_p < RARE_THRESH].tolist())
    if rare_pairs:
        rare = np.isin(pair_id, list(rare_pairs))
        ca = np.where(rare & (ca > 0), np.maximum(ca, KCAP), ca)
        ci = np.where(rare & (ci > 0), np.maximum(ci, KCAP), ci)

    core_bins = []
    for c in range(N_CORES):
        lo, hi = c * NPC, (c + 1) * NPC
        nodes = np.arange(lo, hi)
        binid = ca[lo:hi] * 1024 + ci[lo:hi]
        order = np.argsort(binid, kind="stable")
        nodes_sorted = nodes[order]
        binid_sorted = binid[order]
        uniq, starts = np.unique(binid_sorted, return_index=True)
        ends = np.append(starts[1:], len(binid_sorted))
        bins = {}
        for u, s, e in zip(uniq, starts, ends):
            bins[(int(u) // 1024, int(u) % 1024)] = nodes_sorted[s:e]
        core_bins.append(bins)
    return core_bins


def _plan_v3(all_keys, nrows):
    """Emit the column stream + matmul entries (shared across cores).

    Column stream order: per Ca-group (same Ca, keys sorted):
      act parts (r, q, b) x group cols, then per Ci-run inh parts.
    Each mm entry: (voff, tab, o0, n, block, wcol0, start)
      voff: W column offset; tab: 'a'|'i'; o0: psum col; n: #cols;
      block: 32-lane block index; wcol0: slab col; start: psum overwrite.
    Entries are split at MAX_MM_N / psum-bank (512) / window (WC) boundaries.
    """
    row_off = {}
    off = 0
    for key in all_keys:
        row_off[key] = off
        off += nrows[key]
    NR = off

    mms = []
    sc = 0  # slab column cursor
    GUARD = 16  # unused cols at each window end (no rhs touches the tile edge)

    def emit(parts, tab, g0, g1):
        nonlocal sc
        off_r = 0
        for r, K in enumerate(parts):
            npcol = P // K
            for q in range(K // 4):
                for b in range(4):
                    start = (r == 0) and (q == 0)
                    c = g0
                    while c < g1:
                        if sc % 2:
                            sc += 1  # keep rhs word-aligned
                        n = min(g1 - c, MAX_MM_N, 512 - (c % 512))
                        # never cut a piece at the window boundary: if it
                        # doesn't fit in this window's usable space, move the
                        # whole piece to the next window
                        if (sc % WC) + n > WC - GUARD:
                            sc = (sc // WC + 1) * WC
                        mms.append(
                            (W_OFF[(K, q)], tab, c, n, b, sc, start, K, q, off_r)
                        )
                        sc += n
                        c += n
            off_r += K

    # groups of same Ca
    i = 0
    act_start = None
    while i < len(all_keys):
        j = i
        Ca = all_keys[i][0]
        while j < len(all_keys) and all_keys[j][0] == Ca:
            j += 1
        g0 = row_off[all_keys[i]]
        g1 = row_off[all_keys[j - 1]] + nrows[all_keys[j - 1]]
        if Ca > 0:
            if act_start is None:
                act_start = g0
            emit(_cls_parts(Ca), "a", g0, g1)
        # inh runs within the group
        ii = i
        while ii < j:
            jj = ii
            Ci = all_keys[ii][1]
            while jj < j and all_keys[jj][1] == Ci:
                jj += 1
            if Ci > 0:
                h0 = row_off[all_keys[ii]]
                h1 = row_off[all_keys[jj - 1]] + nrows[all_keys[jj - 1]]
                emit(_cls_parts(Ci), "i", h0, h1)
            ii = jj
        i = j
    if act_start is None:
        act_start = NR

    # reorder pieces bank1-first (stable within each bank, preserving the
    # q0-before-q1 / r0-before-r1 order of every psum region) so the bank-1
    # tail overlaps the bank-0 matmuls; then re-assign slab columns
    mms.sort(key=lambda e: -(e[2] // 512))
    sc = 0
    out = []
    for voff, tab, c, n, b, _sc, start, K, q, off_r in mms:
        if sc % 2:
            sc += 1
        if (sc % WC) + n > WC - GUARD:
            sc = (sc // WC + 1) * WC
        out.append((voff, tab, c, n, b, sc, start, K, q, off_r))
        sc += n
    mms = out

    total_cols = sc
    n_windows = -(-total_cols // WC)
    return mms, total_cols, n_windows, row_off, NR, act_start


def _pack_v3(x, nu_ones, a_src, a_deg, a_ptr, i_src, i_deg, i_ptr, core_bins):
    all_keys = sorted({k for b in core_bins for k in b.keys()})
    nrows = {}
    for key in all_keys:
        nmax = max(len(b.get(key, ())) for b in core_bins)
        nrows[key] = -(-nmax // P)

    mms, total_cols, n_windows, row_off, NR, act_start = _plan_v3(all_keys, nrows)

    # per-core slab construction: iterate the same emission order
    wmat = _build_wmat()
    per_core = []
    meta_orders = []
    for c in range(N_CORES):
        bins = core_bins[c]
        # L grid per bin: flat nr*P, node = L[lane*nr + col]
        Ls = {}
        orders = []
        for key in all_keys:
            nr = nrows[key]
            L = np.full(nr * P, -1, np.int64)
            have = bins.get(key)
            if have is not None:
                L[: len(have)] = have
            Ls[key] = L.reshape(P, nr)  # [lane, col]
            orders.append((key, L))
        meta_orders.append(orders)

        # node grid over global cols: node_at[lane, gcol]
        node_grid = np.concatenate([Ls[key] for key in all_keys], axis=1)
        assert node_grid.shape == (P, NR)

        slab_cols = np.zeros((P, total_cols + (total_cols % 2)), np.float32)
        pp = np.arange(P)
        for voff, tab, o0, n, b, wcol0, start, K, q, off_r in mms:
            src, deg, ptr = (a_src, a_deg, a_ptr) if tab == "a" else (
                i_src, i_deg, i_ptr)
            npcol = P // K
            jj = pp // K
            kk = pp % K
            lanes = 32 * b + q * npcol + jj  # [P]
            nodes = node_grid[lanes, o0 : o0 + n]  # [P, n]
            nd = np.where(nodes >= 0, nodes, 0)
            d = np.where(nodes >= 0, deg[nd], 0)
            eidx = off_r + kk[:, None]
            valid = eidx < d
            gidx = ptr[nd] + np.where(valid, eidx, 0)
            vals = np.where(valid, x[src[gidx]], np.float32(0))
            slab_cols[:, wcol0 : wcol0 + n] = vals

        nvf = np.zeros((P, NR), np.float32)
        valid = node_grid >= 0
        nvf[valid] = x[node_grid[valid]]

        per_core.append(
            {
                "slab": _pack_f16_words(slab_cols[:, : total_cols + (total_cols % 2)]),
                "nodevf": np.ascontiguousarray(nvf),
                "wmat": _pack_f16_words(wmat),
            }
        )

    r00 = None
    if (0, 0) in nrows:
        r00 = (row_off[(0, 0)], nrows[(0, 0)])

    i0_ranges = []
    for key in all_keys:
        if key[1] == 0:
            a, bnd = row_off[key], row_off[key] + nrows[key]
            if i0_ranges and i0_ranges[-1][1] == a:
                i0_ranges[-1] = (i0_ranges[-1][0], bnd)
            else:
                i0_ranges.append((a, bnd))

    shapes = {
        "v3": True,
        "i0_ranges": i0_ranges,
        "keys": all_keys,
        "nrows": nrows,
        "NR": NR,
        "mms": mms,
        "total_cols": total_cols,
        "n_windows": n_windows,
        "act_start": act_start,
        "r00": r00,
    }
    return per_core, meta_orders, shapes


def _build_nc_v3(shapes, loop_R=None, variant="full", debug=False):
    NR = shapes["NR"]
    mms = shapes["mms"]
    n_windows = shapes["n_windows"]
    total_cols = shapes["total_cols"]
    act_start = shapes["act_start"]
    r00 = shapes["r00"]

    n_banks = -(-NR // 512)
    bank_cols = [min(512, NR - 512 * k) for k in range(n_banks)]

    slab_words = (total_cols + 1) // 2
    wmat_words = W_COLS // 2

    nc = bacc.Bacc(None, target_bir_lowering=False)
    sl_d = nc.declare_dram_parameter("slab", [P, slab_words], F32, isOutput=False)
    nvf_d = nc.declare_dram_parameter("nodevf", [P, NR], F32, isOutput=False)
    wm_d = nc.declare_dram_parameter("wmat", [P, wmat_words], F32, isOutput=False)
    out_d = nc.declare_dram_parameter("out", [P, NR], F32, isOutput=True)
    if debug:
        dsq_d = nc.declare_dram_parameter(
            "dbg_sq", [P, slab_words], F32, isOutput=True
        )
        dpa_d = nc.declare_dram_parameter(
            "dbg_pa", [P, 512 * n_banks], F32, isOutput=True
        )
        dpi_d = nc.declare_dram_parameter(
            "dbg_pi", [P, 512 * n_banks], F32, isOutput=True
        )

    MUL = mybir.AluOpType.mult
    ADD = mybir.AluOpType.add

    do_windows = variant not in ("noop", "empty")
    do_sq = variant in ("full", "sq", "notail")
    do_mm = variant in ("full", "notail")
    do_tail = variant == "full"

    with tile.TileContext(nc) as tc:
        with (
            tc.tile_pool(name="slab", bufs=4) as slab_tp,
            tc.tile_pool(name="node", bufs=1) as node_tp,
            tc.tile_pool(name="wm", bufs=1) as wm_tp,
            tc.tile_pool(name="tail", bufs=1) as tail_tp,
            tc.psum_pool(name="ps", bufs=1) as ps_tp,
        ):
            loop_cm = tc.For_i(0, loop_R, 1) if loop_R else contextlib.nullcontext()
            with loop_cm:
                pa = [
                    ps_tp.tile([P, 512], F32, tag=f"pa{k}", name=f"pa{k}")
                    for k in range(n_banks)
                ]
                pi = [
                    ps_tp.tile([P, 512], F32, tag=f"pi{k}", name=f"pi{k}")
                    for k in range(n_banks)
                ]
                pdump = ps_tp.tile([P, 512], F32, tag="pdump", name="pdump")
                ptiles = {"a": pa, "i": pi}

                if variant == "noop":
                    nc.vector.memset(pa[0][:, :1], 0.0)
                    act_start_eff = 0
                else:
                    act_start_eff = act_start
                # init: no-act prefix of asum = 1.0 (num = 1 when no
                # activators); (0,0) bin = 0.0 (agg = 0 when no edges)
                c0 = 0
                while c0 < act_start_eff:
                    k = c0 // 512
                    n = min(act_start_eff - c0, 512 * (k + 1) - c0)
                    nc.vector.memset(pa[k][:, c0 - 512 * k : c0 - 512 * k + n], 1.0)
                    c0 += n
                if r00 is not None and variant != "noop":
                    k = r00[0] // 512
                    assert (r00[0] + r00[1] - 1) // 512 == k
                    nc.vector.memset(
                        pa[k][:, r00[0] - 512 * k : r00[0] - 512 * k + r00[1]], 0.0
                    )
                # isum = 0 only where no inh matmul writes: the (Ka, 0) bins
                # (all other columns are fully overwritten by q0/start=True)
                if variant != "noop":
                    for z0, z1 in shapes["i0_ranges"]:
                        c0 = z0
                        while c0 < z1:
                            k = c0 // 512
                            n = min(z1 - c0, 512 * (k + 1) - c0)
                            nc.vector.memset(
                                pi[k][:, c0 - 512 * k : c0 - 512 * k + n], 0.0
                            )
                            c0 += n

                wm = wm_tp.tile([P, wmat_words], F32, tag="wm")
                wm_f16 = wm[:, :].bitcast(F16)
                nvf = node_tp.tile([P, NR], F32, tag="nvf")
                if variant != "noop":
                    nc.sync.dma_start(out=wm[:, :], in_=wm_d[:, :])
                    nc.scalar.dma_start(out=nvf[:, :], in_=nvf_d[:, :])

                by_win = {}
                for e in mms:
                    by_win.setdefault(e[5] // WC, []).append(e)

                wtiles = []
                for win in range(n_windows) if do_windows else ():
                    wt = slab_tp.tile([P, WINDOW], F32, tag="win")
                    wtiles.append(wt)
                    w0 = win * WINDOW
                    used = min(WINDOW, slab_words - w0)
                    dma_eng = nc.sync
                    half = (used // 2 + 63) // 64 * 64
                    half = min(half, used)
                    dma_eng.dma_start(out=wt[:, :half], in_=sl_d[:, w0 : w0 + half])
                    if half < used:
                        dma_eng.dma_start(
                            out=wt[:, half:used], in_=sl_d[:, w0 + half : w0 + used]
                        )
                    # square in place: ScalarE takes the first chunk, GpSimdE
                    # the rest
                    wf = wt[:, :used].bitcast(F16)
                    ncols = used * 2
                    s_cols = int(ncols * SQ_SPLIT) // 2 * 2
                    if do_sq and s_cols > 0:
                        xs = wf[:, :s_cols]
                        nc.scalar.square(out=xs, in_=xs)
                    if do_sq and s_cols < ncols:
                        xs = wf[:, s_cols:ncols]
                        nc.gpsimd.tensor_tensor(out=xs, in0=xs, in1=xs, op=MUL)

                    if debug:
                        nc.sync.dma_start(
                            out=dsq_d[:, w0 : w0 + used], in_=wt[:, :used]
                        )
                    for voff, tab, o0, n, b, wcol0, start, _K, _q, _ofr in (
                        by_win.get(win, ()) if do_mm else ()
                    ):
                        k = o0 // 512
                        oo = o0 - 512 * k
                        wc = wcol0 - win * WC
                        nc.tensor.matmul(
                            out=ptiles[tab][k][32 * b : 32 * b + 32, oo : oo + n],
                            lhsT=wm_f16[:, voff : voff + 32],
                            rhs=wf[:, wc : wc + n],
                            start=start,
                            stop=True,
                            skip_group_check=True,
                            tile_position=(0, 32 * b),
                        )
                    # sacrificial trailing matmul: the scheduler's window-end
                    # event boundary voids the last matmul's PSUM write on HW;
                    # park it on a dump bank so every real write survives.
                    nc.tensor.matmul(
                        out=pdump[0:32, 0:16],
                        lhsT=wm_f16[:, 0:32],
                        rhs=wf[:, 0:16],
                        start=True,
                        stop=True,
                        skip_group_check=True,
                        tile_position=(0, 0),
                    )

                if debug:
                    dcp = tail_tp.tile([P, 512], F32, tag="dcp")
                    for k in range(n_banks):
                        nc.vector.tensor_copy(out=dcp[:, :], in_=pa[k][:, :])
                        nc.sync.dma_start(
                            out=dpa_d[:, 512 * k : 512 * (k + 1)], in_=dcp[:, :]
                        )
                        nc.vector.tensor_copy(out=dcp[:, :], in_=pi[k][:, :])
                        nc.sync.dma_start(
                            out=dpi_d[:, 512 * k : 512 * (k + 1)], in_=dcp[:, :]
                        )

                # elementwise tail, per psum bank (bank 1 first: its matmuls
                # finish first by plan order, so its tail overlaps bank 0's)
                ot = tail_tp.tile([P, NR], F32, tag="ot")
                den = tail_tp.tile([P, NR], F32, tag="den")
                rde = tail_tp.tile([P, NR], F32, tag="rde")
                if not do_tail and variant != "noop":
                    nc.sync.dma_start(out=out_d[:, :], in_=nvf[:, :])
                for k in reversed(range(n_banks)) if do_tail else ():
                    n = bank_cols[k]
                    dn = den[:, 512 * k : 512 * k + n]
                    rd = rde[:, 512 * k : 512 * k + n]
                    o = ot[:, 512 * k : 512 * k + n]
                    # den = isum + 1 on ScalarE (activation reads PSUM)
                    nc.scalar.add(dn, pi[k][:, :n], 1.0)
                    nc.vector.reciprocal_approx_fast(out=rd, in_=dn)
                    nc.vector.tensor_tensor(out=rd, in0=pa[k][:, :n], in1=rd, op=MUL)
                    # out = agg - x + 1
                    nc.vector.scalar_tensor_tensor(
                        out=o, in0=nvf[:, 512 * k : 512 * k + n], scalar=-1.0,
                        in1=rd, op0=MUL, op1=ADD,
                    )
                    nc.scalar.add(o, o, 1.0)
                    dma_eng = nc.sync if k % 2 == 0 else nc.scalar
                    dma_eng.dma_start(
                        out=out_d[:, 512 * k : 512 * k + n], in_=o
                    )

    nc.finalize()
    return nc


# ================= v2 (DVE) fallback path =================
CHUNK_SLOTS = 4096
V2_WINDOW = 3072


def _degree_classes(max_deg):
    ks = [4, 6, 8, 12, 16, 32]
    while ks[-1] < max_deg:
        ks.append(ks[-1] * 2)
    return ks


def _class_of(deg, ks):
    bounds = np.array(ks)
    idx = np.searchsorted(bounds, deg, side="left")
    out = np.zeros_like(deg)
    nz = deg > 0
    out[nz] = bounds[idx[nz]]
    return out


def _make_plan_v2(all_keys, nrows, fast_k):
    row_off = {}
    off = 0
    for key in all_keys:
        row_off[key] = off
        off += nrows[key]
    total_rows = off

    chunks = []
    act_classes = []
    for key in all_keys:
        if key[0] > 0 and (not act_classes or act_classes[-1][0] != key[0]):
            act_classes.append((key[0], row_off[key]))
    act_seg_rows = {}
    for Ka, seg0 in act_classes:
        seg_rows = sum(nrows[k] for k in all_keys if k[0] == Ka)
        act_seg_rows[Ka] = (seg0, seg_rows)
        T = max(1, CHUNK_SLOTS // Ka)
        r0 = 0
        while r0 < seg_rows:
            t = min(T, seg_rows - r0)
            chunks.append(("a", Ka, seg0 + r0, t))
            r0 += t
    for key in all_keys:
        Ki = key[1]
        if Ki == 0:
            continue
        nr = nrows[key]
        T = max(1, CHUNK_SLOTS // Ki)
        r0 = 0
        while r0 < nr:
            t = min(T, nr - r0)
            chunks.append(("i", Ki, row_off[key] + r0, t))
            r0 += t

    entries = []
    wins = []
    for table, K, g0, t in sorted(chunks, key=lambda c: -(c[3] * c[1])):
        w = t * K
        assert w % 2 == 0
        cw = w // 2 if fast_k else w
        for wi in range(len(wins)):
            if wins[wi] >= cw:
                break
        else:
            wins.append(V2_WINDOW)
            wi = len(wins) - 1
        woff = V2_WINDOW - wins[wi]
        entries.append((table, K, g0, t, wi, woff))
        wins[wi] -= cw
    n_windows = len(wins)
    win_used = [-(-(V2_WINDOW - rem) // 64) * 64 for rem in wins]
    win_start = [0]
    for u in win_used[:-1]:
        win_start.append(win_start[-1] + u)
    return entries, n_windows, win_used, win_start, row_off, act_seg_rows, total_rows


def _pack_v2(x, k_act, k_inh, nu, decay, growth, act_src, act_dst, inh_src, inh_dst,
             fast_k, fast_nodev, tables):
    a_src, a_k, a_deg, a_ptr, i_src, i_k, i_deg, i_ptr = tables

    max_deg = int(max(a_deg.max(), i_deg.max()))
    ks = _degree_classes(max_deg)
    nclasses = len(ks) + 1
    klist = [0] + ks

    ca = _class_of(a_deg, ks)
    ci = _class_of(i_deg, ks)

    kcap = min(16, ks[-1])
    pair_id = ca * 1024 + ci
    uniq_p, cnt_p = np.unique(pair_id, return_counts=True)
    rare_pairs = set(uniq_p[cnt_p < RARE_THRESH].tolist())
    if rare_pairs:
        rare = np.isin(pair_id, list(rare_pairs))
        ca = np.where(rare & (ca > 0), np.maximum(ca, kcap), ca)
        ci = np.where(rare & (ci > 0), np.maximum(ci, kcap), ci)

    core_bins = []
    for c in range(N_CORES):
        lo, hi = c * NPC, (c + 1) * NPC
        nodes = np.arange(lo, hi)
        binid = np.searchsorted(np.array(klist), ca[lo:hi]) * nclasses + np.searchsorted(
            np.array(klist), ci[lo:hi]
        )
        order = np.argsort(binid, kind="stable")
        nodes_sorted = nodes[order]
        binid_sorted = binid[order]
        uniq, starts = np.unique(binid_sorted, return_index=True)
        ends = np.append(starts[1:], len(binid_sorted))
        bins = {}
        for u, s, e in zip(uniq, starts, ends):
            bins[(klist[u // nclasses], klist[u % nclasses])] = nodes_sorted[s:e]
        core_bins.append(bins)

    all_keys = sorted({k for b in core_bins for k in b.keys()})
    nrows = {}
    for key in all_keys:
        nmax = max(len(b.get(key, ())) for b in core_bins)
        nrows[key] = -(-nmax // P)

    (entries, n_windows, win_used, win_start, row_off, act_seg_rows,
     total_rows) = _make_plan_v2(all_keys, nrows, fast_k)

    def build_slab(L, K, rowptr, deg, srcs, kvals, want_k):
        Lc = L.clip(0)
        d = np.where(L >= 0, deg[Lc], 0)
        base = rowptr[Lc]
        cols = np.arange(K)
        idx2 = base[:, None] + cols[None, :]
        valid = cols[None, :] < d[:, None]
        idxc = np.where(valid, idx2, 0)
        sx = np.where(valid, x[srcs[idxc]], np.float32(0)).astype(np.float32)
        sk = None
        if want_k:
            sk = np.where(valid, kvals[idxc], np.float32(0)).astype(np.float32)
        return sx, sk

    per_core = []
    meta_orders = []
    for c in range(N_CORES):
        bins = core_bins[c]
        ax_seg = {}
        ak_seg = {}
        ix_bin = {}
        ik_bin = {}
        xv_l = []
        nodev_l = {"nuv": [], "dev": [], "grv": []}
        orders = []
        ax_parts = {}
        ak_parts = {}
        for key in all_keys:
            Ka, Ki = key
            nr = nrows[key]
            L = np.full(nr * P, -1, np.int64)
            have = bins.get(key)
            if have is not None:
                L[: len(have)] = have
            orders.append((key, L))
            if Ka > 0:
                sx, sk = build_slab(L, Ka, a_ptr, a_deg, a_src, a_k, not fast_k)
                ax_parts.setdefault(Ka, []).append(sx.reshape(P, nr * Ka))
                if not fast_k:
                    ak_parts.setdefault(Ka, []).append(sk.reshape(P, nr * Ka))
            if Ki > 0:
                sx, sk = build_slab(L, Ki, i_ptr, i_deg, i_src, i_k, not fast_k)
                ix_bin[key] = sx.reshape(P, nr * Ki)
                if not fast_k:
                    ik_bin[key] = sk.reshape(P, nr * Ki)
            valid = L >= 0
            Lc = L.clip(0)

            def pk(v):
                return (
                    np.where(valid, v[Lc], np.float32(0))
                    .astype(np.float32)
                    .reshape(P, nr)
                )

            xv_l.append(pk(x))
            if not fast_nodev:
                nodev_l["nuv"].append(pk(nu))
                nodev_l["dev"].append(pk(decay))
                nodev_l["grv"].append(pk(growth))

        for Ka, parts in ax_parts.items():
            ax_seg[Ka] = np.concatenate(parts, axis=1)
            if not fast_k:
                ak_seg[Ka] = np.concatenate(ak_parts[Ka], axis=1)

        slab = np.zeros((P, win_start[-1] + win_used[-1]), np.float32)
        for table, K, g0, t, win, woff in entries:
            w = t * K
            base = win_start[win] + woff
            if table == "a":
                seg0, _ = act_seg_rows[K]
                r0 = g0 - seg0
                sx = ax_seg[K][:, r0 * K : (r0 + t) * K]
                sk = ak_seg[K][:, r0 * K : (r0 + t) * K] if not fast_k else None
            else:
                key = next(
                    kk for kk in all_keys
                    if kk[1] == K and row_off[kk] <= g0 < row_off[kk] + nrows[kk]
                )
                r0 = g0 - row_off[key]
                sx = ix_bin[key][:, r0 * K : (r0 + t) * K]
                sk = ik_bin[key][:, r0 * K : (r0 + t) * K] if not fast_k else None
            xw = w // 2
            slab[:, base : base + xw] = _pack_bf16_words(sx)
            if not fast_k:
                slab[:, base + xw : base + 2 * xw] = _pack_bf16_words(sk)

        core = {
            "slab": slab,
            "nodevf": np.ascontiguousarray(np.concatenate(xv_l, axis=1)),
        }
        if not fast_nodev:
            nodevb = np.concatenate(
                [np.concatenate(nodev_l[nm], axis=1) for nm in ("nuv", "dev", "grv")],
                axis=1,
            )
            core["nodevb"] = _pack_bf16_words(nodevb)
        per_core.append(core)
        meta_orders.append(orders)

    r00 = None
    if (0, 0) in nrows:
        r00 = (row_off[(0, 0)], nrows[(0, 0)])

    shapes = {
        "v3": False,
        "keys": all_keys,
        "nrows": nrows,
        "NR": total_rows,
        "entries": entries,
        "n_windows": n_windows,
        "win_used": win_used,
        "win_start": win_start,
        "fast_k": fast_k,
        "fast_nodev": fast_nodev,
        "r00": r00,
    }
    return per_core, meta_orders, shapes


def _build_nc_v2(shapes, loop_R=None):
    NR = shapes["NR"]
    entries = shapes["entries"]
    n_windows = shapes["n_windows"]
    win_used = shapes["win_used"]
    win_start = shapes["win_start"]
    fast_k = shapes["fast_k"]
    fast_nodev = shapes["fast_nodev"]
    r00 = shapes["r00"]

    NB = (3 * NR + 1) // 2
    nc = bacc.Bacc(None, target_bir_lowering=False)
    sl_d = nc.declare_dram_parameter(
        "slab", [P, win_start[-1] + win_used[-1]], F32, isOutput=False
    )
    nvf_d = nc.declare_dram_parameter("nodevf", [P, NR], F32, isOutput=False)
    nvb_d = None
    if not fast_nodev:
        nvb_d = nc.declare_dram_parameter("nodevb", [P, NB], F32, isOutput=False)
    out_d = nc.declare_dram_parameter("out", [P, NR], F32, isOutput=True)

    MUL = mybir.AluOpType.mult
    ADD = mybir.AluOpType.add
    X = mybir.AxisListType.X

    with tile.TileContext(nc) as tc:
        with (
            tc.tile_pool(name="slab", bufs=3) as slab_tp,
            tc.tile_pool(name="sums", bufs=1) as sums_tp,
            tc.tile_pool(name="node", bufs=1) as node_tp,
        ):
            loop_cm = tc.For_i(0, loop_R, 1) if loop_R else contextlib.nullcontext()
            with loop_cm:
                asum = sums_tp.tile([P, NR], F32, tag="asum")
                isum = sums_tp.tile([P, NR], F32, tag="isum")
                nc.vector.memset(asum[:, :], 1.0)
                nc.vector.memset(isum[:, :], 0.0)
                if r00 is not None:
                    nc.vector.memset(asum[:, r00[0] : r00[0] + r00[1]], 0.0)
                bufs = {"a": asum, "i": isum}

                nvf = node_tp.tile([P, NR], F32, tag="nvf")
                nc.scalar.dma_start(out=nvf[:, :], in_=nvf_d[:, :])
                if not fast_nodev:
                    nvb = node_tp.tile([P, NB], F32, tag="nvb")
                    nc.scalar.dma_start(out=nvb[:, :], in_=nvb_d[:, :])
                    nvb_b = nvb[:, :].bitcast(BF16)
                    iv = {}
                    for j, nm in enumerate(("nuv", "dev", "grv")):
                        iv[nm] = nvb_b[:, j * NR : (j + 1) * NR]

                by_win = {}
                for e in entries:
                    by_win.setdefault(e[4], []).append(e)
                sq_engine = 0
                for win in range(n_windows):
                    wt = slab_tp.tile([P, V2_WINDOW], F32, tag="win")
                    used = win_used[win]
                    half = (used // 2 + 63) // 64 * 64
                    half = min(half, used)
                    dma_eng = nc.sync
                    dma_eng.dma_start(
                        out=wt[:, :half],
                        in_=sl_d[:, win_start[win] : win_start[win] + half],
                    )
                    if half < used:
                        dma_eng.dma_start(
                            out=wt[:, half:used],
                            in_=sl_d[:, win_start[win] + half : win_start[win] + used],
                        )
                    for table, K, g0, t, _win, woff in by_win.get(win, ()):
                        w = t * K
                        xw = w // 2
                        xs = wt[:, woff : woff + xw].bitcast(BF16)
                        if fast_k:
                            if sq_engine == 0:
                                nc.scalar.square(out=xs, in_=xs)
                            else:
                                nc.gpsimd.tensor_tensor(out=xs, in0=xs, in1=xs, op=MUL)
                            sq_engine ^= 1
                        else:
                            kS = wt[:, woff + xw : woff + 2 * xw].bitcast(BF16)
                            nc.scalar.square(out=xs, in_=xs)
                            nc.gpsimd.tensor_tensor(out=xs, in0=xs, in1=kS, op=MUL)
                        nc.vector.tensor_reduce(
                            out=bufs[table][:, g0 : g0 + t],
                            in_=xs.rearrange("p (t k) -> p t k", k=K),
                            axis=X,
                            op=ADD,
                        )

                den = node_tp.tile([P, NR], F32, tag="den")
                rde = node_tp.tile([P, NR], F32, tag="rde")
                scr = node_tp.tile([P, NR], F32, tag="scr")
                ot = node_tp.tile([P, NR], F32, tag="ot")
                A = lambda tl: tl[:, :]

                nc.vector.tensor_scalar_add(A(den), A(isum), 1.0)
                nc.vector.reciprocal_approx_accurate(
                    out=A(rde), in_=A(den), scratch=A(scr)
                )
                nc.vector.tensor_tensor(out=A(rde), in0=A(asum), in1=A(rde), op=MUL)
                if fast_nodev:
                    nc.vector.scalar_tensor_tensor(
                        out=A(ot), in0=A(nvf), scalar=-1.0, in1=A(rde),
                        op0=MUL, op1=ADD,
                    )
                    nc.vector.tensor_scalar_add(A(ot), A(ot), 1.0)
                else:
                    nc.vector.tensor_tensor(out=A(ot), in0=iv["nuv"], in1=A(rde), op=MUL)
                    nc.vector.scalar_tensor_tensor(
                        out=A(scr), in0=iv["dev"], scalar=-1.0, in1=A(nvf),
                        op0=MUL, op1=MUL,
                    )
                    nc.vector.tensor_tensor(out=A(ot), in0=A(ot), in1=A(scr), op=ADD)
                    nc.vector.tensor_tensor(out=A(ot), in0=A(ot), in1=iv["grv"], op=ADD)
                nc.scalar.dma_start(out=out_d[:, :], in_=ot[:, :])

    nc.finalize()
    return nc


# ================= dispatchers =================
def _pack(x, k_act, k_inh, nu, decay, growth, act_src, act_dst, inh_src, inh_dst):
    fast_k = bool(np.all(k_act == 1.0) and np.all(k_inh == 1.0))
    fast_nodev = bool(
        np.all(nu == 1.0) and np.all(decay == 1.0) and np.all(growth == 1.0)
    )

    def sorted_table(src, dst, k):
        order = np.argsort(dst, kind="stable")
        deg = np.bincount(dst, minlength=N_NODES).astype(np.int64)
        rowptr = np.zeros(N_NODES + 1, np.int64)
        np.cumsum(deg, out=rowptr[1:])
        return src[order], k[order], deg, rowptr

    a_src, a_k, a_deg, a_ptr = sorted_table(act_src, act_dst, k_act)
    i_src, i_k, i_deg, i_ptr = sorted_table(inh_src, inh_dst, k_inh)

    if fast_k and fast_nodev:
        core_bins = _make_bins(a_deg, i_deg)
        return _pack_v3(
            x, None, a_src, a_deg, a_ptr, i_src, i_deg, i_ptr, core_bins
        )
    return _pack_v2(
        x, k_act, k_inh, nu, decay, growth, act_src, act_dst, inh_src, inh_dst,
        fast_k, fast_nodev,
        (a_src, a_k, a_deg, a_ptr, i_src, i_k, i_deg, i_ptr),
    )


def _build_nc(shapes, loop_R=None, variant="full"):
    if shapes.get("v3"):
        return _build_nc_v3(shapes, loop_R=loop_R, variant=variant)
    return _build_nc_v2(shapes, loop_R=loop_R)


def kernel(**inputs) -> np.ndarray:
    per_core, meta_orders, shapes = _pack(
        np.asarray(inputs["x"], np.float32),
        np.asarray(inputs["k_act"], np.float32),
        np.asarray(inputs["k_inh"], np.float32),
        np.asarray(inputs["nu"], np.float32),
        np.asarray(inputs["decay"], np.float32),
        np.asarray(inputs["growth"], np.float32),
        np.asarray(inputs["act_src"]),
        np.asarray(inputs["act_dst"]),
        np.asarray(inputs["inh_src"]),
        np.asarray(inputs["inh_dst"]),
    )
    nc = _build_nc(shapes)
    in_maps = [dict(per_core[c]) for c in range(N_CORES)]
    res = run_bass_kernel_spmd(nc, in_maps, list(range(N_CORES)))

    out_full = np.zeros(N_NODES, np.float32)
    nrows = shapes["nrows"]
    for c in range(N_CORES):
        arr = res.results[c]["out"]
        offN = 0
        for key, L in meta_orders[c]:
            nr = nrows[key]
            block = arr[:, offN : offN + nr].reshape(P * nr)
            valid = L >= 0
            out_full[L[valid]] = block[valid]
            offN += nr
    return out_full


# revision 62
# speedup vs baseline: 1.3618x; 1.1672x over previous
"""BioGNN message-passing kernel for 8 trn2 NeuronCores.

Strategy (sharding chosen per the "you choose" contract):
  - Shard by DESTINATION node range: core c owns nodes [c*125k, (c+1)*125k).
    Each edge is routed (host-side layout) to the core owning its dst, so no
    all-reduce is needed; the host concatenates per-core output slices.
  - Host does LAYOUT ONLY: per owned node, incoming edges are padded into
    dense ELL slabs binned by in-degree class; each slot carries a copy of
    x[src] in fp16. Node order inside a core is a host-known permutation
    (bin-major); outputs are un-permuted on the host.
  - Device does ALL arithmetic. v3 fast path (all-ones gains/node params):
    slabs are laid out as 128-slot COLUMNS (K edge slots x 128//K nodes per
    column); ScalarE/GpSimdE square the fp16 slabs in place; the TENSOR
    engine computes the per-node segment sums as block-diagonal ones-weight
    matmuls into PSUM (out = W.T @ x^2-slab, one 32-lane out block per
    matmul, stripes accumulate with start=False); VectorE runs only the
    elementwise tail per PSUM bank (1/(1+inh_sum), ratio, decay/growth).
    Degree classes {4,8,12,16,24,32,48,...} decompose into {16,8,4} passes
    that accumulate into the same PSUM element.
  - Mask-free tail: the no-act column prefix of asum is memset to 1.0 and
    matmuls OVERWRITE (start=True) columns of nodes that own act slots,
    reproducing `where(has_act, act_sum, 1)`; the (no-act, no-inh) bin
    columns are memset to 0 so `agg` lands at 0 there. Class promotion of
    rare bins never crosses the zero boundary, keeping both tricks exact.
  - v2 fallback path (general gains/node params) keeps the DVE
    square/multiply/tensor_reduce pipeline with bf16 slabs.
"""

import contextlib

import ml_dtypes
import numpy as np

import concourse.bacc as bacc
import concourse.mybir as mybir
import concourse.tile as tile
from concourse.bass_utils import run_bass_kernel_spmd

N_NODES = 1_000_000
N_CORES = 8
NPC = N_NODES // N_CORES
P = 128

F32 = mybir.dt.float32
BF16 = mybir.dt.bfloat16
F16 = mybir.dt.float16

# ---------------- v3 (tensor-engine) parameters ----------------
WINDOW = 2048        # slab window width per partition in f32 words
WC = WINDOW * 2      # fp16 slab columns per window
RARE_THRESH = 16384  # global node count below which a (ca, ci) pair is promoted
KCAP = 16
MAX_MM_N = 512       # moving-operand free-dim cap
SQ_SPLIT = 0.55      # fraction of window squares on ScalarE (rest GpSimdE)

# class value -> decomposition into base parts (descending)
def _cls_parts(v):
    parts = []
    while v >= 16:
        parts.append(16)
        v -= 16
    if v >= 8:
        parts.append(8)
        v -= 8
    if v >= 4:
        parts.append(4)
        v -= 4
    assert v == 0
    return parts


CLS_VALUES = [4, 8, 12, 16, 24, 32, 48, 64, 96, 128]
# W variant table: (K, q) -> column offset (in 32-col units) inside wmat
W_VARIANTS = [(4, 0), (8, 0), (8, 1), (16, 0), (16, 1), (16, 2), (16, 3)]
W_OFF = {kq: 32 * i for i, kq in enumerate(W_VARIANTS)}
W_COLS = 32 * len(W_VARIANTS)


def _class_of_v3(deg):
    bounds = np.array(CLS_VALUES)
    idx = np.searchsorted(bounds, deg, side="left")
    out = np.zeros_like(deg)
    nz = deg > 0
    out[nz] = bounds[np.minimum(idx[nz], len(bounds) - 1)]
    return out


def _build_wmat():
    w = np.zeros((P, W_COLS), np.float32)
    for (K, q), off in W_OFF.items():
        npcol = P // K
        p = np.arange(P)
        m = q * npcol + p // K
        w[p, off + m] = 1.0
    return w


def _pack_f16_words(arr):
    """[P, n] f32 -> [P, ceil(n/2)] f32 words holding round-to-nearest fp16."""
    a = arr.astype(np.float16)
    if a.shape[1] % 2:
        a = np.concatenate([a, np.zeros((a.shape[0], 1), np.float16)], axis=1)
    u = a.view(np.uint16)
    w = (u[:, 0::2].astype(np.uint32) | (u[:, 1::2].astype(np.uint32) << 16)).view(
        np.float32
    )
    return np.ascontiguousarray(w)


def _pack_bf16_words(arr):
    a = arr.astype(ml_dtypes.bfloat16)
    if a.shape[1] % 2:
        a = np.concatenate([a, np.zeros((a.shape[0], 1), ml_dtypes.bfloat16)], axis=1)
    u = a.view(np.uint16)
    w = (u[:, 0::2].astype(np.uint32) | (u[:, 1::2].astype(np.uint32) << 16)).view(
        np.float32
    )
    return np.ascontiguousarray(w)


def _make_bins(a_deg, i_deg):
    ca = _class_of_v3(a_deg)
    ci = _class_of_v3(i_deg)
    pair_id = ca * 1024 + ci
    uniq_p, cnt_p = np.unique(pair_id, return_counts=True)
    rare_pairs = set(uniq_p[cnt_p < RARE_THRESH].tolist())
    if rare_pairs:
        rare = np.isin(pair_id, list(rare_pairs))
        ca = np.where(rare & (ca > 0), np.maximum(ca, KCAP), ca)
        ci = np.where(rare & (ci > 0), np.maximum(ci, KCAP), ci)

    core_bins = []
    for c in range(N_CORES):
        lo, hi = c * NPC, (c + 1) * NPC
        nodes = np.arange(lo, hi)
        binid = ca[lo:hi] * 1024 + ci[lo:hi]
        order = np.argsort(binid, kind="stable")
        nodes_sorted = nodes[order]
        binid_sorted = binid[order]
        uniq, starts = np.unique(binid_sorted, return_index=True)
        ends = np.append(starts[1:], len(binid_sorted))
        bins = {}
        for u, s, e in zip(uniq, starts, ends):
            bins[(int(u) // 1024, int(u) % 1024)] = nodes_sorted[s:e]
        core_bins.append(bins)
    return core_bins


def _plan_v3(all_keys, nrows):
    """Emit the column stream + matmul entries (shared across cores).

    Column stream order: per Ca-group (same Ca, keys sorted):
      act parts (r, q, b) x group cols, then per Ci-run inh parts.
    Each mm entry: (voff, tab, o0, n, block, wcol0, start)
      voff: W column offset; tab: 'a'|'i'; o0: psum col; n: #cols;
      block: 32-lane block index; wcol0: slab col; start: psum overwrite.
    Entries are split at MAX_MM_N / psum-bank (512) / window (WC) boundaries.
    """
    row_off = {}
    off = 0
    for key in all_keys:
        row_off[key] = off
        off += nrows[key]
    NR = off

    mms = []
    sc = 0  # slab column cursor
    GUARD = 16  # unused cols at each window end (no rhs touches the tile edge)

    def emit(parts, tab, g0, g1):
        nonlocal sc
        off_r = 0
        for r, K in enumerate(parts):
            npcol = P // K
            for q in range(K // 4):
                for b in range(4):
                    start = (r == 0) and (q == 0)
                    c = g0
                    while c < g1:
                        if sc % 2:
                            sc += 1  # keep rhs word-aligned
                        n = min(g1 - c, MAX_MM_N, 512 - (c % 512))
                        # never cut a piece at the window boundary: if it
                        # doesn't fit in this window's usable space, move the
                        # whole piece to the next window
                        if (sc % WC) + n > WC - GUARD:
                            sc = (sc // WC + 1) * WC
                        mms.append(
                            (W_OFF[(K, q)], tab, c, n, b, sc, start, K, q, off_r)
                        )
                        sc += n
                        c += n
            off_r += K

    # groups of same Ca
    i = 0
    act_start = None
    while i < len(all_keys):
        j = i
        Ca = all_keys[i][0]
        while j < len(all_keys) and all_keys[j][0] == Ca:
            j += 1
        g0 = row_off[all_keys[i]]
        g1 = row_off[all_keys[j - 1]] + nrows[all_keys[j - 1]]
        if Ca > 0:
            if act_start is None:
                act_start = g0
            emit(_cls_parts(Ca), "a", g0, g1)
        # inh runs within the group
        ii = i
        while ii < j:
            jj = ii
            Ci = all_keys[ii][1]
            while jj < j and all_keys[jj][1] == Ci:
                jj += 1
            if Ci > 0:
                h0 = row_off[all_keys[ii]]
                h1 = row_off[all_keys[jj - 1]] + nrows[all_keys[jj - 1]]
                emit(_cls_parts(Ci), "i", h0, h1)
            ii = jj
        i = j
    if act_start is None:
        act_start = NR

    # reorder pieces bank1-first (stable within each bank, preserving the
    # q0-before-q1 / r0-before-r1 order of every psum region) so the bank-1
    # tail overlaps the bank-0 matmuls; then re-assign slab columns
    mms.sort(key=lambda e: -(e[2] // 512))
    sc = 0
    out = []
    for voff, tab, c, n, b, _sc, start, K, q, off_r in mms:
        if sc % 2:
            sc += 1
        if (sc % WC) + n > WC - GUARD:
            sc = (sc // WC + 1) * WC
        out.append((voff, tab, c, n, b, sc, start, K, q, off_r))
        sc += n
    mms = out

    total_cols = sc
    n_windows = -(-total_cols // WC)
    return mms, total_cols, n_windows, row_off, NR, act_start


def _pack_v3(x, nu_ones, a_src, a_deg, a_ptr, i_src, i_deg, i_ptr, core_bins):
    all_keys = sorted({k for b in core_bins for k in b.keys()})
    nrows = {}
    for key in all_keys:
        nmax = max(len(b.get(key, ())) for b in core_bins)
        nrows[key] = -(-nmax // P)

    mms, total_cols, n_windows, row_off, NR, act_start = _plan_v3(all_keys, nrows)

    # per-core slab construction: iterate the same emission order
    wmat = _build_wmat()
    per_core = []
    meta_orders = []
    for c in range(N_CORES):
        bins = core_bins[c]
        # L grid per bin: flat nr*P, node = L[lane*nr + col]
        Ls = {}
        orders = []
        for key in all_keys:
            nr = nrows[key]
            L = np.full(nr * P, -1, np.int64)
            have = bins.get(key)
            if have is not None:
                L[: len(have)] = have
            Ls[key] = L.reshape(P, nr)  # [lane, col]
            orders.append((key, L))
        meta_orders.append(orders)

        # node grid over global cols: node_at[lane, gcol]
        node_grid = np.concatenate([Ls[key] for key in all_keys], axis=1)
        assert node_grid.shape == (P, NR)

        slab_cols = np.zeros((P, total_cols + (total_cols % 2)), np.float32)
        pp = np.arange(P)
        for voff, tab, o0, n, b, wcol0, start, K, q, off_r in mms:
            src, deg, ptr = (a_src, a_deg, a_ptr) if tab == "a" else (
                i_src, i_deg, i_ptr)
            npcol = P // K
            jj = pp // K
            kk = pp % K
            lanes = 32 * b + q * npcol + jj  # [P]
            nodes = node_grid[lanes, o0 : o0 + n]  # [P, n]
            nd = np.where(nodes >= 0, nodes, 0)
            d = np.where(nodes >= 0, deg[nd], 0)
            eidx = off_r + kk[:, None]
            valid = eidx < d
            gidx = ptr[nd] + np.where(valid, eidx, 0)
            vals = np.where(valid, x[src[gidx]], np.float32(0))
            slab_cols[:, wcol0 : wcol0 + n] = vals

        nvf = np.zeros((P, NR), np.float32)
        valid = node_grid >= 0
        nvf[valid] = x[node_grid[valid]]

        per_core.append(
            {
                "slab": _pack_f16_words(slab_cols[:, : total_cols + (total_cols % 2)]),
                "nodevf": np.ascontiguousarray(nvf),
                "wmat": _pack_f16_words(wmat),
            }
        )

    r00 = None
    if (0, 0) in nrows:
        r00 = (row_off[(0, 0)], nrows[(0, 0)])

    i0_ranges = []
    for key in all_keys:
        if key[1] == 0:
            a, bnd = row_off[key], row_off[key] + nrows[key]
            if i0_ranges and i0_ranges[-1][1] == a:
                i0_ranges[-1] = (i0_ranges[-1][0], bnd)
            else:
                i0_ranges.append((a, bnd))

    shapes = {
        "v3": True,
        "i0_ranges": i0_ranges,
        "keys": all_keys,
        "nrows": nrows,
        "NR": NR,
        "mms": mms,
        "total_cols": total_cols,
        "n_windows": n_windows,
        "act_start": act_start,
        "r00": r00,
    }
    return per_core, meta_orders, shapes


def _build_nc_v3(shapes, loop_R=None, variant="full", debug=False):
    NR = shapes["NR"]
    mms = shapes["mms"]
    n_windows = shapes["n_windows"]
    total_cols = shapes["total_cols"]
    act_start = shapes["act_start"]
    r00 = shapes["r00"]

    n_banks = -(-NR // 512)
    bank_cols = [min(512, NR - 512 * k) for k in range(n_banks)]

    slab_words = (total_cols + 1) // 2
    wmat_words = W_COLS // 2

    nc = bacc.Bacc(None, target_bir_lowering=False)
    sl_d = nc.declare_dram_parameter("slab", [P, slab_words], F32, isOutput=False)
    nvf_d = nc.declare_dram_parameter("nodevf", [P, NR], F32, isOutput=False)
    wm_d = nc.declare_dram_parameter("wmat", [P, wmat_words], F32, isOutput=False)
    out_d = nc.declare_dram_parameter("out", [P, NR], BF16, isOutput=True)
    if debug:
        dsq_d = nc.declare_dram_parameter(
            "dbg_sq", [P, slab_words], F32, isOutput=True
        )
        dpa_d = nc.declare_dram_parameter(
            "dbg_pa", [P, 512 * n_banks], F32, isOutput=True
        )
        dpi_d = nc.declare_dram_parameter(
            "dbg_pi", [P, 512 * n_banks], F32, isOutput=True
        )

    MUL = mybir.AluOpType.mult
    ADD = mybir.AluOpType.add

    do_windows = variant not in ("noop", "empty")
    do_sq = variant in ("full", "sq", "notail")
    do_mm = variant in ("full", "notail")
    do_tail = variant == "full"

    with tile.TileContext(nc) as tc:
        with (
            tc.tile_pool(name="slab", bufs=4) as slab_tp,
            tc.tile_pool(name="node", bufs=1) as node_tp,
            tc.tile_pool(name="wm", bufs=1) as wm_tp,
            tc.tile_pool(name="tail", bufs=1) as tail_tp,
            tc.psum_pool(name="ps", bufs=1) as ps_tp,
        ):
            loop_cm = tc.For_i(0, loop_R, 1) if loop_R else contextlib.nullcontext()
            with loop_cm:
                pa = [
                    ps_tp.tile([P, 512], F32, tag=f"pa{k}", name=f"pa{k}")
                    for k in range(n_banks)
                ]
                pi = [
                    ps_tp.tile([P, 512], F32, tag=f"pi{k}", name=f"pi{k}")
                    for k in range(n_banks)
                ]
                pdump = ps_tp.tile([P, 512], F32, tag="pdump", name="pdump")
                ptiles = {"a": pa, "i": pi}

                if variant == "noop":
                    nc.vector.memset(pa[0][:, :1], 0.0)
                    act_start_eff = 0
                else:
                    act_start_eff = act_start
                # init: no-act prefix of asum = 1.0 (num = 1 when no
                # activators); (0,0) bin = 0.0 (agg = 0 when no edges)
                c0 = 0
                while c0 < act_start_eff:
                    k = c0 // 512
                    n = min(act_start_eff - c0, 512 * (k + 1) - c0)
                    nc.vector.memset(pa[k][:, c0 - 512 * k : c0 - 512 * k + n], 1.0)
                    c0 += n
                if r00 is not None and variant != "noop":
                    k = r00[0] // 512
                    assert (r00[0] + r00[1] - 1) // 512 == k
                    nc.vector.memset(
                        pa[k][:, r00[0] - 512 * k : r00[0] - 512 * k + r00[1]], 0.0
                    )
                # isum = 0 only where no inh matmul writes: the (Ka, 0) bins
                # (all other columns are fully overwritten by q0/start=True)
                if variant != "noop":
                    for z0, z1 in shapes["i0_ranges"]:
                        c0 = z0
                        while c0 < z1:
                            k = c0 // 512
                            n = min(z1 - c0, 512 * (k + 1) - c0)
                            nc.vector.memset(
                                pi[k][:, c0 - 512 * k : c0 - 512 * k + n], 0.0
                            )
                            c0 += n

                wm = wm_tp.tile([P, wmat_words], F32, tag="wm")
                wm_f16 = wm[:, :].bitcast(F16)
                nvf = node_tp.tile([P, NR], F32, tag="nvf")
                if variant != "noop":
                    nc.sync.dma_start(out=wm[:, :], in_=wm_d[:, :])
                    nc.scalar.dma_start(out=nvf[:, :], in_=nvf_d[:, :])

                by_win = {}
                for e in mms:
                    by_win.setdefault(e[5] // WC, []).append(e)

                wtiles = []
                for win in range(n_windows) if do_windows else ():
                    wt = slab_tp.tile([P, WINDOW], F32, tag="win")
                    wtiles.append(wt)
                    w0 = win * WINDOW
                    used = min(WINDOW, slab_words - w0)
                    dma_eng = nc.sync if win % 2 == 0 else nc.scalar
                    half = (used // 2 + 63) // 64 * 64
                    half = min(half, used)
                    wf = wt[:, :used].bitcast(F16)
                    # DMA + square per half so matmuls start on the first
                    # half while the second streams in
                    for h0, h1 in ((0, half), (half, used)):
                        if h0 >= h1:
                            continue
                        dma_eng.dma_start(
                            out=wt[:, h0:h1], in_=sl_d[:, w0 + h0 : w0 + h1]
                        )
                        if do_sq:
                            c0, c1 = 2 * h0, 2 * h1
                            s_cols = c0 + int((c1 - c0) * SQ_SPLIT) // 2 * 2
                            if s_cols > c0:
                                xs = wf[:, c0:s_cols]
                                nc.scalar.square(out=xs, in_=xs)
                            if s_cols < c1:
                                xs = wf[:, s_cols:c1]
                                nc.gpsimd.tensor_tensor(out=xs, in0=xs, in1=xs, op=MUL)

                    if debug:
                        nc.sync.dma_start(
                            out=dsq_d[:, w0 : w0 + used], in_=wt[:, :used]
                        )
                    for voff, tab, o0, n, b, wcol0, start, _K, _q, _ofr in (
                        by_win.get(win, ()) if do_mm else ()
                    ):
                        k = o0 // 512
                        oo = o0 - 512 * k
                        wc = wcol0 - win * WC
                        nc.tensor.matmul(
                            out=ptiles[tab][k][32 * b : 32 * b + 32, oo : oo + n],
                            lhsT=wm_f16[:, voff : voff + 32],
                            rhs=wf[:, wc : wc + n],
                            start=start,
                            stop=True,
                            skip_group_check=True,
                            tile_position=(0, 32 * b),
                        )
                    # sacrificial trailing matmul: the scheduler's window-end
                    # event boundary voids the last matmul's PSUM write on HW;
                    # park it on a dump bank so every real write survives.
                    nc.tensor.matmul(
                        out=pdump[0:32, 0:16],
                        lhsT=wm_f16[:, 0:32],
                        rhs=wf[:, 0:16],
                        start=True,
                        stop=True,
                        skip_group_check=True,
                        tile_position=(0, 0),
                    )

                if debug:
                    dcp = tail_tp.tile([P, 512], F32, tag="dcp")
                    for k in range(n_banks):
                        nc.vector.tensor_copy(out=dcp[:, :], in_=pa[k][:, :])
                        nc.sync.dma_start(
                            out=dpa_d[:, 512 * k : 512 * (k + 1)], in_=dcp[:, :]
                        )
                        nc.vector.tensor_copy(out=dcp[:, :], in_=pi[k][:, :])
                        nc.sync.dma_start(
                            out=dpi_d[:, 512 * k : 512 * (k + 1)], in_=dcp[:, :]
                        )

                # elementwise tail, per psum bank (bank 1 first: its matmuls
                # finish first by plan order, so its tail overlaps bank 0's)
                ot = tail_tp.tile([P, NR], F32, tag="ot")
                den = tail_tp.tile([P, NR], F32, tag="den")
                rde = tail_tp.tile([P, NR], F32, tag="rde")
                obf = tail_tp.tile([P, NR], BF16, tag="obf")
                if not do_tail and variant != "noop":
                    nc.vector.memset(obf[:, :], 0.0)
                    nc.sync.dma_start(out=out_d[:, :], in_=obf[:, :])
                for k in reversed(range(n_banks)) if do_tail else ():
                    n = bank_cols[k]
                    dn = den[:, 512 * k : 512 * k + n]
                    rd = rde[:, 512 * k : 512 * k + n]
                    o = ot[:, 512 * k : 512 * k + n]
                    # den = isum + 1 on ScalarE (activation reads PSUM)
                    nc.scalar.add(dn, pi[k][:, :n], 1.0)
                    nc.vector.reciprocal_approx_fast(out=rd, in_=dn)
                    nc.vector.tensor_tensor(out=rd, in0=pa[k][:, :n], in1=rd, op=MUL)
                    # out = agg - x + 1
                    nc.vector.scalar_tensor_tensor(
                        out=o, in0=nvf[:, 512 * k : 512 * k + n], scalar=-1.0,
                        in1=rd, op0=MUL, op1=ADD,
                    )
                    ob = obf[:, 512 * k : 512 * k + n]
                    nc.scalar.add(ob, o, 1.0)
                    dma_eng = nc.sync if k % 2 == 0 else nc.scalar
                    dma_eng.dma_start(
                        out=out_d[:, 512 * k : 512 * k + n], in_=ob
                    )

    nc.finalize()
    return nc


# ================= v2 (DVE) fallback path =================
CHUNK_SLOTS = 4096
V2_WINDOW = 2048


def _degree_classes(max_deg):
    ks = [4, 6, 8, 12, 16, 32]
    while ks[-1] < max_deg:
        ks.append(ks[-1] * 2)
    return ks


def _class_of(deg, ks):
    bounds = np.array(ks)
    idx = np.searchsorted(bounds, deg, side="left")
    out = np.zeros_like(deg)
    nz = deg > 0
    out[nz] = bounds[idx[nz]]
    return out


def _make_plan_v2(all_keys, nrows, fast_k):
    row_off = {}
    off = 0
    for key in all_keys:
        row_off[key] = off
        off += nrows[key]
    total_rows = off

    chunks = []
    act_classes = []
    for key in all_keys:
        if key[0] > 0 and (not act_classes or act_classes[-1][0] != key[0]):
            act_classes.append((key[0], row_off[key]))
    act_seg_rows = {}
    for Ka, seg0 in act_classes:
        seg_rows = sum(nrows[k] for k in all_keys if k[0] == Ka)
        act_seg_rows[Ka] = (seg0, seg_rows)
        T = max(1, CHUNK_SLOTS // Ka)
        r0 = 0
        while r0 < seg_rows:
            t = min(T, seg_rows - r0)
            chunks.append(("a", Ka, seg0 + r0, t))
            r0 += t
    for key in all_keys:
        Ki = key[1]
        if Ki == 0:
            continue
        nr = nrows[key]
        T = max(1, CHUNK_SLOTS // Ki)
        r0 = 0
        while r0 < nr:
            t = min(T, nr - r0)
            chunks.append(("i", Ki, row_off[key] + r0, t))
            r0 += t

    entries = []
    wins = []
    for table, K, g0, t in sorted(chunks, key=lambda c: -(c[3] * c[1])):
        w = t * K
        assert w % 2 == 0
        cw = w // 2 if fast_k else w
        for wi in range(len(wins)):
            if wins[wi] >= cw:
                break
        else:
            wins.append(V2_WINDOW)
            wi = len(wins) - 1
        woff = V2_WINDOW - wins[wi]
        entries.append((table, K, g0, t, wi, woff))
        wins[wi] -= cw
    n_windows = len(wins)
    win_used = [-(-(V2_WINDOW - rem) // 64) * 64 for rem in wins]
    win_start = [0]
    for u in win_used[:-1]:
        win_start.append(win_start[-1] + u)
    return entries, n_windows, win_used, win_start, row_off, act_seg_rows, total_rows


def _pack_v2(x, k_act, k_inh, nu, decay, growth, act_src, act_dst, inh_src, inh_dst,
             fast_k, fast_nodev, tables):
    a_src, a_k, a_deg, a_ptr, i_src, i_k, i_deg, i_ptr = tables

    max_deg = int(max(a_deg.max(), i_deg.max()))
    ks = _degree_classes(max_deg)
    nclasses = len(ks) + 1
    klist = [0] + ks

    ca = _class_of(a_deg, ks)
    ci = _class_of(i_deg, ks)

    kcap = min(16, ks[-1])
    pair_id = ca * 1024 + ci
    uniq_p, cnt_p = np.unique(pair_id, return_counts=True)
    rare_pairs = set(uniq_p[cnt_p < RARE_THRESH].tolist())
    if rare_pairs:
        rare = np.isin(pair_id, list(rare_pairs))
        ca = np.where(rare & (ca > 0), np.maximum(ca, kcap), ca)
        ci = np.where(rare & (ci > 0), np.maximum(ci, kcap), ci)

    core_bins = []
    for c in range(N_CORES):
        lo, hi = c * NPC, (c + 1) * NPC
        nodes = np.arange(lo, hi)
        binid = np.searchsorted(np.array(klist), ca[lo:hi]) * nclasses + np.searchsorted(
            np.array(klist), ci[lo:hi]
        )
        order = np.argsort(binid, kind="stable")
        nodes_sorted = nodes[order]
        binid_sorted = binid[order]
        uniq, starts = np.unique(binid_sorted, return_index=True)
        ends = np.append(starts[1:], len(binid_sorted))
        bins = {}
        for u, s, e in zip(uniq, starts, ends):
            bins[(klist[u // nclasses], klist[u % nclasses])] = nodes_sorted[s:e]
        core_bins.append(bins)

    all_keys = sorted({k for b in core_bins for k in b.keys()})
    nrows = {}
    for key in all_keys:
        nmax = max(len(b.get(key, ())) for b in core_bins)
        nrows[key] = -(-nmax // P)

    (entries, n_windows, win_used, win_start, row_off, act_seg_rows,
     total_rows) = _make_plan_v2(all_keys, nrows, fast_k)

    def build_slab(L, K, rowptr, deg, srcs, kvals, want_k):
        Lc = L.clip(0)
        d = np.where(L >= 0, deg[Lc], 0)
        base = rowptr[Lc]
        cols = np.arange(K)
        idx2 = base[:, None] + cols[None, :]
        valid = cols[None, :] < d[:, None]
        idxc = np.where(valid, idx2, 0)
        sx = np.where(valid, x[srcs[idxc]], np.float32(0)).astype(np.float32)
        sk = None
        if want_k:
            sk = np.where(valid, kvals[idxc], np.float32(0)).astype(np.float32)
        return sx, sk

    per_core = []
    meta_orders = []
    for c in range(N_CORES):
        bins = core_bins[c]
        ax_seg = {}
        ak_seg = {}
        ix_bin = {}
        ik_bin = {}
        xv_l = []
        nodev_l = {"nuv": [], "dev": [], "grv": []}
        orders = []
        ax_parts = {}
        ak_parts = {}
        for key in all_keys:
            Ka, Ki = key
            nr = nrows[key]
            L = np.full(nr * P, -1, np.int64)
            have = bins.get(key)
            if have is not None:
                L[: len(have)] = have
            orders.append((key, L))
            if Ka > 0:
                sx, sk = build_slab(L, Ka, a_ptr, a_deg, a_src, a_k, not fast_k)
                ax_parts.setdefault(Ka, []).append(sx.reshape(P, nr * Ka))
                if not fast_k:
                    ak_parts.setdefault(Ka, []).append(sk.reshape(P, nr * Ka))
            if Ki > 0:
                sx, sk = build_slab(L, Ki, i_ptr, i_deg, i_src, i_k, not fast_k)
                ix_bin[key] = sx.reshape(P, nr * Ki)
                if not fast_k:
                    ik_bin[key] = sk.reshape(P, nr * Ki)
            valid = L >= 0
            Lc = L.clip(0)

            def pk(v):
                return (
                    np.where(valid, v[Lc], np.float32(0))
                    .astype(np.float32)
                    .reshape(P, nr)
                )

            xv_l.append(pk(x))
            if not fast_nodev:
                nodev_l["nuv"].append(pk(nu))
                nodev_l["dev"].append(pk(decay))
                nodev_l["grv"].append(pk(growth))

        for Ka, parts in ax_parts.items():
            ax_seg[Ka] = np.concatenate(parts, axis=1)
            if not fast_k:
                ak_seg[Ka] = np.concatenate(ak_parts[Ka], axis=1)

        slab = np.zeros((P, win_start[-1] + win_used[-1]), np.float32)
        for table, K, g0, t, win, woff in entries:
            w = t * K
            base = win_start[win] + woff
            if table == "a":
                seg0, _ = act_seg_rows[K]
                r0 = g0 - seg0
                sx = ax_seg[K][:, r0 * K : (r0 + t) * K]
                sk = ak_seg[K][:, r0 * K : (r0 + t) * K] if not fast_k else None
            else:
                key = next(
                    kk for kk in all_keys
                    if kk[1] == K and row_off[kk] <= g0 < row_off[kk] + nrows[kk]
                )
                r0 = g0 - row_off[key]
                sx = ix_bin[key][:, r0 * K : (r0 + t) * K]
                sk = ik_bin[key][:, r0 * K : (r0 + t) * K] if not fast_k else None
            xw = w // 2
            slab[:, base : base + xw] = _pack_bf16_words(sx)
            if not fast_k:
                slab[:, base + xw : base + 2 * xw] = _pack_bf16_words(sk)

        core = {
            "slab": slab,
            "nodevf": np.ascontiguousarray(np.concatenate(xv_l, axis=1)),
        }
        if not fast_nodev:
            nodevb = np.concatenate(
                [np.concatenate(nodev_l[nm], axis=1) for nm in ("nuv", "dev", "grv")],
                axis=1,
            )
            core["nodevb"] = _pack_bf16_words(nodevb)
        per_core.append(core)
        meta_orders.append(orders)

    r00 = None
    if (0, 0) in nrows:
        r00 = (row_off[(0, 0)], nrows[(0, 0)])

    shapes = {
        "v3": False,
        "keys": all_keys,
        "nrows": nrows,
        "NR": total_rows,
        "entries": entries,
        "n_windows": n_windows,
        "win_used": win_used,
        "win_start": win_start,
        "fast_k": fast_k,
        "fast_nodev": fast_nodev,
        "r00": r00,
    }
    return per_core, meta_orders, shapes


def _build_nc_v2(shapes, loop_R=None):
    NR = shapes["NR"]
    entries = shapes["entries"]
    n_windows = shapes["n_windows"]
    win_used = shapes["win_used"]
    win_start = shapes["win_start"]
    fast_k = shapes["fast_k"]
    fast_nodev = shapes["fast_nodev"]
    r00 = shapes["r00"]

    NB = (3 * NR + 1) // 2
    nc = bacc.Bacc(None, target_bir_lowering=False)
    sl_d = nc.declare_dram_parameter(
        "slab", [P, win_start[-1] + win_used[-1]], F32, isOutput=False
    )
    nvf_d = nc.declare_dram_parameter("nodevf", [P, NR], F32, isOutput=False)
    nvb_d = None
    if not fast_nodev:
        nvb_d = nc.declare_dram_parameter("nodevb", [P, NB], F32, isOutput=False)
    out_d = nc.declare_dram_parameter("out", [P, NR], F32, isOutput=True)

    MUL = mybir.AluOpType.mult
    ADD = mybir.AluOpType.add
    X = mybir.AxisListType.X

    with tile.TileContext(nc) as tc:
        with (
            tc.tile_pool(name="slab", bufs=3) as slab_tp,
            tc.tile_pool(name="sums", bufs=1) as sums_tp,
            tc.tile_pool(name="node", bufs=1) as node_tp,
        ):
            loop_cm = tc.For_i(0, loop_R, 1) if loop_R else contextlib.nullcontext()
            with loop_cm:
                asum = sums_tp.tile([P, NR], F32, tag="asum")
                isum = sums_tp.tile([P, NR], F32, tag="isum")
                nc.vector.memset(asum[:, :], 1.0)
                nc.vector.memset(isum[:, :], 0.0)
                if r00 is not None:
                    nc.vector.memset(asum[:, r00[0] : r00[0] + r00[1]], 0.0)
                bufs = {"a": asum, "i": isum}

                nvf = node_tp.tile([P, NR], F32, tag="nvf")
                nc.scalar.dma_start(out=nvf[:, :], in_=nvf_d[:, :])
                if not fast_nodev:
                    nvb = node_tp.tile([P, NB], F32, tag="nvb")
                    nc.scalar.dma_start(out=nvb[:, :], in_=nvb_d[:, :])
                    nvb_b = nvb[:, :].bitcast(BF16)
                    iv = {}
                    for j, nm in enumerate(("nuv", "dev", "grv")):
                        iv[nm] = nvb_b[:, j * NR : (j + 1) * NR]

                by_win = {}
                for e in entries:
                    by_win.setdefault(e[4], []).append(e)
                sq_engine = 0
                for win in range(n_windows):
                    wt = slab_tp.tile([P, V2_WINDOW], F32, tag="win")
                    used = win_used[win]
                    half = (used // 2 + 63) // 64 * 64
                    half = min(half, used)
                    dma_eng = nc.sync
                    dma_eng.dma_start(
                        out=wt[:, :half],
                        in_=sl_d[:, win_start[win] : win_start[win] + half],
                    )
                    if half < used:
                        dma_eng.dma_start(
                            out=wt[:, half:used],
                            in_=sl_d[:, win_start[win] + half : win_start[win] + used],
                        )
                    for table, K, g0, t, _win, woff in by_win.get(win, ()):
                        w = t * K
                        xw = w // 2
                        xs = wt[:, woff : woff + xw].bitcast(BF16)
                        if fast_k:
                            if sq_engine == 0:
                                nc.scalar.square(out=xs, in_=xs)
                            else:
                                nc.gpsimd.tensor_tensor(out=xs, in0=xs, in1=xs, op=MUL)
                            sq_engine ^= 1
                        else:
                            kS = wt[:, woff + xw : woff + 2 * xw].bitcast(BF16)
                            nc.scalar.square(out=xs, in_=xs)
                            nc.gpsimd.tensor_tensor(out=xs, in0=xs, in1=kS, op=MUL)
                        nc.vector.tensor_reduce(
                            out=bufs[table][:, g0 : g0 + t],
                            in_=xs.rearrange("p (t k) -> p t k", k=K),
                            axis=X,
                            op=ADD,
                        )

                den = node_tp.tile([P, NR], F32, tag="den")
                rde = node_tp.tile([P, NR], F32, tag="rde")
                scr = node_tp.tile([P, NR], F32, tag="scr")
                ot = node_tp.tile([P, NR], F32, tag="ot")
                A = lambda tl: tl[:, :]

                nc.vector.tensor_scalar_add(A(den), A(isum), 1.0)
                nc.vector.reciprocal_approx_accurate(
                    out=A(rde), in_=A(den), scratch=A(scr)
                )
                nc.vector.tensor_tensor(out=A(rde), in0=A(asum), in1=A(rde), op=MUL)
                if fast_nodev:
                    nc.vector.scalar_tensor_tensor(
                        out=A(ot), in0=A(nvf), scalar=-1.0, in1=A(rde),
                        op0=MUL, op1=ADD,
                    )
                    nc.vector.tensor_scalar_add(A(ot), A(ot), 1.0)
                else:
                    nc.vector.tensor_tensor(out=A(ot), in0=iv["nuv"], in1=A(rde), op=MUL)
                    nc.vector.scalar_tensor_tensor(
                        out=A(scr), in0=iv["dev"], scalar=-1.0, in1=A(nvf),
                        op0=MUL, op1=MUL,
                    )
                    nc.vector.tensor_tensor(out=A(ot), in0=A(ot), in1=A(scr), op=ADD)
                    nc.vector.tensor_tensor(out=A(ot), in0=A(ot), in1=iv["grv"], op=ADD)
                nc.scalar.dma_start(out=out_d[:, :], in_=ot[:, :])

    nc.finalize()
    return nc


# ================= dispatchers =================
def _pack(x, k_act, k_inh, nu, decay, growth, act_src, act_dst, inh_src, inh_dst):
    fast_k = bool(np.all(k_act == 1.0) and np.all(k_inh == 1.0))
    fast_nodev = bool(
        np.all(nu == 1.0) and np.all(decay == 1.0) and np.all(growth == 1.0)
    )

    def sorted_table(src, dst, k):
        order = np.argsort(dst, kind="stable")
        deg = np.bincount(dst, minlength=N_NODES).astype(np.int64)
        rowptr = np.zeros(N_NODES + 1, np.int64)
        np.cumsum(deg, out=rowptr[1:])
        return src[order], k[order], deg, rowptr

    a_src, a_k, a_deg, a_ptr = sorted_table(act_src, act_dst, k_act)
    i_src, i_k, i_deg, i_ptr = sorted_table(inh_src, inh_dst, k_inh)

    if fast_k and fast_nodev:
        core_bins = _make_bins(a_deg, i_deg)
        return _pack_v3(
            x, None, a_src, a_deg, a_ptr, i_src, i_deg, i_ptr, core_bins
        )
    return _pack_v2(
        x, k_act, k_inh, nu, decay, growth, act_src, act_dst, inh_src, inh_dst,
        fast_k, fast_nodev,
        (a_src, a_k, a_deg, a_ptr, i_src, i_k, i_deg, i_ptr),
    )


def _build_nc(shapes, loop_R=None, variant="full"):
    if shapes.get("v3"):
        return _build_nc_v3(shapes, loop_R=loop_R, variant=variant)
    return _build_nc_v2(shapes, loop_R=loop_R)


def kernel(**inputs) -> np.ndarray:
    per_core, meta_orders, shapes = _pack(
        np.asarray(inputs["x"], np.float32),
        np.asarray(inputs["k_act"], np.float32),
        np.asarray(inputs["k_inh"], np.float32),
        np.asarray(inputs["nu"], np.float32),
        np.asarray(inputs["decay"], np.float32),
        np.asarray(inputs["growth"], np.float32),
        np.asarray(inputs["act_src"]),
        np.asarray(inputs["act_dst"]),
        np.asarray(inputs["inh_src"]),
        np.asarray(inputs["inh_dst"]),
    )
    nc = _build_nc(shapes)
    in_maps = [dict(per_core[c]) for c in range(N_CORES)]
    res = run_bass_kernel_spmd(nc, in_maps, list(range(N_CORES)))

    out_full = np.zeros(N_NODES, np.float32)
    nrows = shapes["nrows"]
    for c in range(N_CORES):
        arr = np.asarray(res.results[c]["out"]).astype(np.float32)
        offN = 0
        for key, L in meta_orders[c]:
            nr = nrows[key]
            block = arr[:, offN : offN + nr].reshape(P * nr)
            valid = L >= 0
            out_full[L[valid]] = block[valid]
            offN += nr
    return out_full


# revision 67
# speedup vs baseline: 1.4019x; 1.0294x over previous
"""BioGNN message-passing kernel for 8 trn2 NeuronCores.

Strategy (sharding chosen per the "you choose" contract):
  - Shard by DESTINATION node range: core c owns nodes [c*125k, (c+1)*125k).
    Each edge is routed (host-side layout) to the core owning its dst, so no
    all-reduce is needed; the host concatenates per-core output slices.
  - Host does LAYOUT ONLY: per owned node, incoming edges are padded into
    dense ELL slabs binned by in-degree class; each slot carries a copy of
    x[src] in fp16. Node order inside a core is a host-known permutation
    (bin-major); outputs are un-permuted on the host.
  - Device does ALL arithmetic. v3 fast path (all-ones gains/node params):
    slabs are laid out as 128-slot COLUMNS (K edge slots x 128//K nodes per
    column); ScalarE/GpSimdE square the fp16 slabs in place; the TENSOR
    engine computes the per-node segment sums as block-diagonal ones-weight
    matmuls into PSUM (out = W.T @ x^2-slab, one 32-lane out block per
    matmul, stripes accumulate with start=False); VectorE runs only the
    elementwise tail per PSUM bank (1/(1+inh_sum), ratio, decay/growth).
    Degree classes {4,8,12,16,24,32,48,...} decompose into {16,8,4} passes
    that accumulate into the same PSUM element.
  - Mask-free tail: the no-act column prefix of asum is memset to 1.0 and
    matmuls OVERWRITE (start=True) columns of nodes that own act slots,
    reproducing `where(has_act, act_sum, 1)`; the (no-act, no-inh) bin
    columns are memset to 0 so `agg` lands at 0 there. Class promotion of
    rare bins never crosses the zero boundary, keeping both tricks exact.
  - v2 fallback path (general gains/node params) keeps the DVE
    square/multiply/tensor_reduce pipeline with bf16 slabs.
"""

import contextlib

import ml_dtypes
import numpy as np

import concourse.bacc as bacc
import concourse.mybir as mybir
import concourse.tile as tile
from concourse.bass_utils import run_bass_kernel_spmd

N_NODES = 1_000_000
N_CORES = 8
NPC = N_NODES // N_CORES
P = 128

F32 = mybir.dt.float32
BF16 = mybir.dt.bfloat16
F16 = mybir.dt.float16

# ---------------- v3 (tensor-engine) parameters ----------------
WINDOW = 2048        # slab window width per partition in f32 words
WC = WINDOW * 2      # fp16 slab columns per window
RARE_THRESH = 16384  # global node count below which a (ca, ci) pair is promoted
KCAP = 16
MAX_MM_N = 512       # moving-operand free-dim cap
SQ_SPLIT = 0.72      # fraction of window squares on ScalarE (rest GpSimdE)

# class value -> decomposition into base parts (descending)
def _cls_parts(v):
    parts = []
    while v >= 16:
        parts.append(16)
        v -= 16
    if v >= 8:
        parts.append(8)
        v -= 8
    if v >= 4:
        parts.append(4)
        v -= 4
    assert v == 0
    return parts


CLS_VALUES = [4, 8, 12, 16, 24, 32, 48, 64, 96, 128]
# W variant table: (K, q) -> column offset (in 32-col units) inside wmat
W_VARIANTS = [(4, 0), (8, 0), (8, 1), (16, 0), (16, 1), (16, 2), (16, 3)]
W_OFF = {kq: 32 * i for i, kq in enumerate(W_VARIANTS)}
W_COLS = 32 * len(W_VARIANTS)


def _class_of_v3(deg):
    bounds = np.array(CLS_VALUES)
    idx = np.searchsorted(bounds, deg, side="left")
    out = np.zeros_like(deg)
    nz = deg > 0
    out[nz] = bounds[np.minimum(idx[nz], len(bounds) - 1)]
    return out


def _build_wmat():
    w = np.zeros((P, W_COLS), np.float32)
    for (K, q), off in W_OFF.items():
        npcol = P // K
        p = np.arange(P)
        m = q * npcol + p // K
        w[p, off + m] = 1.0
    return w


def _pack_f16_words(arr):
    """[P, n] f32 -> [P, ceil(n/2)] f32 words holding round-to-nearest fp16."""
    a = arr.astype(np.float16)
    if a.shape[1] % 2:
        a = np.concatenate([a, np.zeros((a.shape[0], 1), np.float16)], axis=1)
    u = a.view(np.uint16)
    w = (u[:, 0::2].astype(np.uint32) | (u[:, 1::2].astype(np.uint32) << 16)).view(
        np.float32
    )
    return np.ascontiguousarray(w)


def _pack_bf16_words(arr):
    a = arr.astype(ml_dtypes.bfloat16)
    if a.shape[1] % 2:
        a = np.concatenate([a, np.zeros((a.shape[0], 1), ml_dtypes.bfloat16)], axis=1)
    u = a.view(np.uint16)
    w = (u[:, 0::2].astype(np.uint32) | (u[:, 1::2].astype(np.uint32) << 16)).view(
        np.float32
    )
    return np.ascontiguousarray(w)


def _make_bins(a_deg, i_deg):
    ca = _class_of_v3(a_deg)
    ci = _class_of_v3(i_deg)
    pair_id = ca * 1024 + ci
    uniq_p, cnt_p = np.unique(pair_id, return_counts=True)
    rare_pairs = set(uniq_p[cnt_p < RARE_THRESH].tolist())
    if rare_pairs:
        rare = np.isin(pair_id, list(rare_pairs))
        ca = np.where(rare & (ca > 0), np.maximum(ca, KCAP), ca)
        ci = np.where(rare & (ci > 0), np.maximum(ci, KCAP), ci)

    core_bins = []
    for c in range(N_CORES):
        lo, hi = c * NPC, (c + 1) * NPC
        nodes = np.arange(lo, hi)
        binid = ca[lo:hi] * 1024 + ci[lo:hi]
        order = np.argsort(binid, kind="stable")
        nodes_sorted = nodes[order]
        binid_sorted = binid[order]
        uniq, starts = np.unique(binid_sorted, return_index=True)
        ends = np.append(starts[1:], len(binid_sorted))
        bins = {}
        for u, s, e in zip(uniq, starts, ends):
            bins[(int(u) // 1024, int(u) % 1024)] = nodes_sorted[s:e]
        core_bins.append(bins)
    return core_bins


def _plan_v3(all_keys, nrows):
    """Emit the column stream + matmul entries (shared across cores).

    Column stream order: per Ca-group (same Ca, keys sorted):
      act parts (r, q, b) x group cols, then per Ci-run inh parts.
    Each mm entry: (voff, tab, o0, n, block, wcol0, start)
      voff: W column offset; tab: 'a'|'i'; o0: psum col; n: #cols;
      block: 32-lane block index; wcol0: slab col; start: psum overwrite.
    Entries are split at MAX_MM_N / psum-bank (512) / window (WC) boundaries.
    """
    row_off = {}
    off = 0
    for key in all_keys:
        row_off[key] = off
        off += nrows[key]
    NR = off

    mms = []
    sc = 0  # slab column cursor
    GUARD = 16  # unused cols at each window end (no rhs touches the tile edge)

    def emit(parts, tab, g0, g1):
        nonlocal sc
        off_r = 0
        for r, K in enumerate(parts):
            npcol = P // K
            for q in range(K // 4):
                for b in range(4):
                    start = (r == 0) and (q == 0)
                    c = g0
                    while c < g1:
                        if sc % 2:
                            sc += 1  # keep rhs word-aligned
                        n = min(g1 - c, MAX_MM_N, 512 - (c % 512))
                        # never cut a piece at the window boundary: if it
                        # doesn't fit in this window's usable space, move the
                        # whole piece to the next window
                        if (sc % WC) + n > WC - GUARD:
                            sc = (sc // WC + 1) * WC
                        mms.append(
                            (W_OFF[(K, q)], tab, c, n, b, sc, start, K, q, off_r)
                        )
                        sc += n
                        c += n
            off_r += K

    # groups of same Ca
    i = 0
    act_start = None
    while i < len(all_keys):
        j = i
        Ca = all_keys[i][0]
        while j < len(all_keys) and all_keys[j][0] == Ca:
            j += 1
        g0 = row_off[all_keys[i]]
        g1 = row_off[all_keys[j - 1]] + nrows[all_keys[j - 1]]
        if Ca > 0:
            if act_start is None:
                act_start = g0
            emit(_cls_parts(Ca), "a", g0, g1)
        # inh runs within the group
        ii = i
        while ii < j:
            jj = ii
            Ci = all_keys[ii][1]
            while jj < j and all_keys[jj][1] == Ci:
                jj += 1
            if Ci > 0:
                h0 = row_off[all_keys[ii]]
                h1 = row_off[all_keys[jj - 1]] + nrows[all_keys[jj - 1]]
                emit(_cls_parts(Ci), "i", h0, h1)
            ii = jj
        i = j
    if act_start is None:
        act_start = NR

    # reorder pieces bank1-first (stable within each bank, preserving the
    # q0-before-q1 / r0-before-r1 order of every psum region) so the bank-1
    # tail overlaps the bank-0 matmuls; then re-assign slab columns
    mms.sort(key=lambda e: -(e[2] // 512))
    sc = 0
    out = []
    for voff, tab, c, n, b, _sc, start, K, q, off_r in mms:
        if sc % 2:
            sc += 1
        if (sc % WC) + n > WC - GUARD:
            sc = (sc // WC + 1) * WC
        out.append((voff, tab, c, n, b, sc, start, K, q, off_r))
        sc += n
    mms = out

    total_cols = sc
    n_windows = -(-total_cols // WC)
    return mms, total_cols, n_windows, row_off, NR, act_start


def _pack_v3(x, nu_ones, a_src, a_deg, a_ptr, i_src, i_deg, i_ptr, core_bins):
    all_keys = sorted({k for b in core_bins for k in b.keys()})
    nrows = {}
    for key in all_keys:
        nmax = max(len(b.get(key, ())) for b in core_bins)
        nrows[key] = -(-nmax // P)

    mms, total_cols, n_windows, row_off, NR, act_start = _plan_v3(all_keys, nrows)

    # per-core slab construction: iterate the same emission order
    wmat = _build_wmat()
    per_core = []
    meta_orders = []
    for c in range(N_CORES):
        bins = core_bins[c]
        # L grid per bin: flat nr*P, node = L[lane*nr + col]
        Ls = {}
        orders = []
        for key in all_keys:
            nr = nrows[key]
            L = np.full(nr * P, -1, np.int64)
            have = bins.get(key)
            if have is not None:
                L[: len(have)] = have
            Ls[key] = L.reshape(P, nr)  # [lane, col]
            orders.append((key, L))
        meta_orders.append(orders)

        # node grid over global cols: node_at[lane, gcol]
        node_grid = np.concatenate([Ls[key] for key in all_keys], axis=1)
        assert node_grid.shape == (P, NR)

        slab_cols = np.zeros((P, total_cols + (total_cols % 2)), np.float32)
        pp = np.arange(P)
        for voff, tab, o0, n, b, wcol0, start, K, q, off_r in mms:
            src, deg, ptr = (a_src, a_deg, a_ptr) if tab == "a" else (
                i_src, i_deg, i_ptr)
            npcol = P // K
            jj = pp // K
            kk = pp % K
            lanes = 32 * b + q * npcol + jj  # [P]
            nodes = node_grid[lanes, o0 : o0 + n]  # [P, n]
            nd = np.where(nodes >= 0, nodes, 0)
            d = np.where(nodes >= 0, deg[nd], 0)
            eidx = off_r + kk[:, None]
            valid = eidx < d
            gidx = ptr[nd] + np.where(valid, eidx, 0)
            vals = np.where(valid, x[src[gidx]], np.float32(0))
            slab_cols[:, wcol0 : wcol0 + n] = vals

        nvf = np.zeros((P, NR), np.float32)
        valid = node_grid >= 0
        nvf[valid] = x[node_grid[valid]]

        per_core.append(
            {
                "slab": _pack_f16_words(slab_cols[:, : total_cols + (total_cols % 2)]),
                "nodevf": np.ascontiguousarray(nvf),
                "wmat": _pack_f16_words(wmat),
            }
        )

    r00 = None
    if (0, 0) in nrows:
        r00 = (row_off[(0, 0)], nrows[(0, 0)])

    i0_ranges = []
    for key in all_keys:
        if key[1] == 0:
            a, bnd = row_off[key], row_off[key] + nrows[key]
            if i0_ranges and i0_ranges[-1][1] == a:
                i0_ranges[-1] = (i0_ranges[-1][0], bnd)
            else:
                i0_ranges.append((a, bnd))

    shapes = {
        "v3": True,
        "i0_ranges": i0_ranges,
        "keys": all_keys,
        "nrows": nrows,
        "NR": NR,
        "mms": mms,
        "total_cols": total_cols,
        "n_windows": n_windows,
        "act_start": act_start,
        "r00": r00,
    }
    return per_core, meta_orders, shapes


def _build_nc_v3(shapes, loop_R=None, variant="full", debug=False):
    NR = shapes["NR"]
    mms = shapes["mms"]
    n_windows = shapes["n_windows"]
    total_cols = shapes["total_cols"]
    act_start = shapes["act_start"]
    r00 = shapes["r00"]

    n_banks = -(-NR // 512)
    bank_cols = [min(512, NR - 512 * k) for k in range(n_banks)]

    slab_words = (total_cols + 1) // 2
    wmat_words = W_COLS // 2

    nc = bacc.Bacc(None, target_bir_lowering=False)
    sl_d = nc.declare_dram_parameter("slab", [P, slab_words], F32, isOutput=False)
    nvf_d = nc.declare_dram_parameter("nodevf", [P, NR], F32, isOutput=False)
    wm_d = nc.declare_dram_parameter("wmat", [P, wmat_words], F32, isOutput=False)
    out_d = nc.declare_dram_parameter("out", [P, NR], BF16, isOutput=True)
    if debug:
        dsq_d = nc.declare_dram_parameter(
            "dbg_sq", [P, slab_words], F32, isOutput=True
        )
        dpa_d = nc.declare_dram_parameter(
            "dbg_pa", [P, 512 * n_banks], F32, isOutput=True
        )
        dpi_d = nc.declare_dram_parameter(
            "dbg_pi", [P, 512 * n_banks], F32, isOutput=True
        )

    MUL = mybir.AluOpType.mult
    ADD = mybir.AluOpType.add

    do_windows = variant not in ("noop", "empty")
    do_sq = variant in ("full", "sq", "notail")
    if variant == "mmnosq":
        do_sq = False
    do_mm = variant in ("full", "notail", "mmnosq")
    do_tail = variant == "full"

    with tile.TileContext(nc) as tc:
        with (
            tc.tile_pool(name="slab", bufs=6) as slab_tp,
            tc.tile_pool(name="node", bufs=2) as node_tp,
            tc.tile_pool(name="wm", bufs=2) as wm_tp,
            tc.tile_pool(name="tail", bufs=2) as tail_tp,
            tc.psum_pool(name="ps", bufs=1) as ps_tp,
        ):
            loop_cm = tc.For_i(0, loop_R, 1) if loop_R else contextlib.nullcontext()
            with loop_cm:
                pa = [
                    ps_tp.tile([P, 512], F32, tag=f"pa{k}", name=f"pa{k}")
                    for k in range(n_banks)
                ]
                pi = [
                    ps_tp.tile([P, 512], F32, tag=f"pi{k}", name=f"pi{k}")
                    for k in range(n_banks)
                ]
                pdump = ps_tp.tile([P, 512], F32, tag="pdump", name="pdump")
                ptiles = {"a": pa, "i": pi}

                if variant == "noop":
                    nc.vector.memset(pa[0][:, :1], 0.0)
                    act_start_eff = 0
                else:
                    act_start_eff = act_start
                # init: no-act prefix of asum = 1.0 (num = 1 when no
                # activators); (0,0) bin = 0.0 (agg = 0 when no edges)
                c0 = 0
                while c0 < act_start_eff:
                    k = c0 // 512
                    n = min(act_start_eff - c0, 512 * (k + 1) - c0)
                    nc.vector.memset(pa[k][:, c0 - 512 * k : c0 - 512 * k + n], 1.0)
                    c0 += n
                if r00 is not None and variant != "noop":
                    k = r00[0] // 512
                    assert (r00[0] + r00[1] - 1) // 512 == k
                    nc.vector.memset(
                        pa[k][:, r00[0] - 512 * k : r00[0] - 512 * k + r00[1]], 0.0
                    )
                # isum = 0 only where no inh matmul writes: the (Ka, 0) bins
                # (all other columns are fully overwritten by q0/start=True)
                if variant != "noop":
                    for z0, z1 in shapes["i0_ranges"]:
                        c0 = z0
                        while c0 < z1:
                            k = c0 // 512
                            n = min(z1 - c0, 512 * (k + 1) - c0)
                            nc.vector.memset(
                                pi[k][:, c0 - 512 * k : c0 - 512 * k + n], 0.0
                            )
                            c0 += n

                wm = wm_tp.tile([P, wmat_words], F32, tag="wm")
                wm_f16 = wm[:, :].bitcast(F16)
                nvf = node_tp.tile([P, NR], F32, tag="nvf")
                if variant != "noop":
                    nc.sync.dma_start(out=wm[:, :], in_=wm_d[:, :])
                    nc.scalar.dma_start(out=nvf[:, :], in_=nvf_d[:, :])

                by_win = {}
                for e in mms:
                    by_win.setdefault(e[5] // WC, []).append(e)

                wtiles = []
                for win in range(n_windows) if do_windows else ():
                    wt = slab_tp.tile([P, WINDOW], F32, tag="win")
                    wtiles.append(wt)
                    w0 = win * WINDOW
                    used = min(WINDOW, slab_words - w0)
                    dma_eng = nc.sync if win % 2 == 0 else nc.scalar
                    half = (used // 2 + 63) // 64 * 64
                    half = min(half, used)
                    wf = wt[:, :used].bitcast(F16)
                    # DMA + square per half so matmuls start on the first
                    # half while the second streams in
                    for h0, h1 in ((0, half), (half, used)):
                        if h0 >= h1:
                            continue
                        dma_eng.dma_start(
                            out=wt[:, h0:h1], in_=sl_d[:, w0 + h0 : w0 + h1]
                        )
                        if do_sq:
                            c0, c1 = 2 * h0, 2 * h1
                            s_cols = c0 + int((c1 - c0) * SQ_SPLIT) // 2 * 2
                            if s_cols > c0:
                                xs = wf[:, c0:s_cols]
                                nc.scalar.square(out=xs, in_=xs)
                            if s_cols < c1:
                                xs = wf[:, s_cols:c1]
                                nc.gpsimd.tensor_tensor(out=xs, in0=xs, in1=xs, op=MUL)

                    if debug:
                        nc.sync.dma_start(
                            out=dsq_d[:, w0 : w0 + used], in_=wt[:, :used]
                        )
                    for voff, tab, o0, n, b, wcol0, start, _K, _q, _ofr in (
                        by_win.get(win, ()) if do_mm else ()
                    ):
                        k = o0 // 512
                        oo = o0 - 512 * k
                        wc = wcol0 - win * WC
                        nc.tensor.matmul(
                            out=ptiles[tab][k][32 * b : 32 * b + 32, oo : oo + n],
                            lhsT=wm_f16[:, voff : voff + 32],
                            rhs=wf[:, wc : wc + n],
                            start=start,
                            stop=True,
                            skip_group_check=True,
                            tile_position=(0, 32 * b),
                        )
                    # sacrificial trailing matmul: the scheduler's window-end
                    # event boundary voids the last matmul's PSUM write on HW;
                    # park it on a dump bank so every real write survives.
                    nc.tensor.matmul(
                        out=pdump[0:32, 0:16],
                        lhsT=wm_f16[:, 0:32],
                        rhs=wf[:, 0:16],
                        start=True,
                        stop=True,
                        skip_group_check=True,
                        tile_position=(0, 0),
                    )

                if debug:
                    dcp = tail_tp.tile([P, 512], F32, tag="dcp")
                    for k in range(n_banks):
                        nc.vector.tensor_copy(out=dcp[:, :], in_=pa[k][:, :])
                        nc.sync.dma_start(
                            out=dpa_d[:, 512 * k : 512 * (k + 1)], in_=dcp[:, :]
                        )
                        nc.vector.tensor_copy(out=dcp[:, :], in_=pi[k][:, :])
                        nc.sync.dma_start(
                            out=dpi_d[:, 512 * k : 512 * (k + 1)], in_=dcp[:, :]
                        )

                # elementwise tail, per psum bank (bank 1 first: its matmuls
                # finish first by plan order, so its tail overlaps bank 0's)
                ot = tail_tp.tile([P, NR], F32, tag="ot")
                den = tail_tp.tile([P, NR], F32, tag="den")
                rde = tail_tp.tile([P, NR], F32, tag="rde")
                obf = tail_tp.tile([P, NR], BF16, tag="obf")
                if not do_tail and variant != "noop":
                    nc.vector.memset(obf[:, :], 0.0)
                    nc.sync.dma_start(out=out_d[:, :], in_=obf[:, :])
                for k in reversed(range(n_banks)) if do_tail else ():
                    n = bank_cols[k]
                    dn = den[:, 512 * k : 512 * k + n]
                    rd = rde[:, 512 * k : 512 * k + n]
                    o = ot[:, 512 * k : 512 * k + n]
                    # den = isum + 1 on ScalarE (activation reads PSUM)
                    nc.scalar.add(dn, pi[k][:, :n], 1.0)
                    nc.vector.reciprocal_approx_fast(out=rd, in_=dn)
                    nc.vector.tensor_tensor(out=rd, in0=pa[k][:, :n], in1=rd, op=MUL)
                    # out = agg - x + 1
                    nc.vector.scalar_tensor_tensor(
                        out=o, in0=nvf[:, 512 * k : 512 * k + n], scalar=-1.0,
                        in1=rd, op0=MUL, op1=ADD,
                    )
                    ob = obf[:, 512 * k : 512 * k + n]
                    nc.scalar.add(ob, o, 1.0)
                    dma_eng = nc.sync if k % 2 == 0 else nc.scalar
                    dma_eng.dma_start(
                        out=out_d[:, 512 * k : 512 * k + n], in_=ob
                    )

    nc.finalize()
    return nc


# ================= v2 (DVE) fallback path =================
CHUNK_SLOTS = 4096
V2_WINDOW = 2048


def _degree_classes(max_deg):
    ks = [4, 6, 8, 12, 16, 32]
    while ks[-1] < max_deg:
        ks.append(ks[-1] * 2)
    return ks


def _class_of(deg, ks):
    bounds = np.array(ks)
    idx = np.searchsorted(bounds, deg, side="left")
    out = np.zeros_like(deg)
    nz = deg > 0
    out[nz] = bounds[idx[nz]]
    return out


def _make_plan_v2(all_keys, nrows, fast_k):
    row_off = {}
    off = 0
    for key in all_keys:
        row_off[key] = off
        off += nrows[key]
    total_rows = off

    chunks = []
    act_classes = []
    for key in all_keys:
        if key[0] > 0 and (not act_classes or act_classes[-1][0] != key[0]):
            act_classes.append((key[0], row_off[key]))
    act_seg_rows = {}
    for Ka, seg0 in act_classes:
        seg_rows = sum(nrows[k] for k in all_keys if k[0] == Ka)
        act_seg_rows[Ka] = (seg0, seg_rows)
        T = max(1, CHUNK_SLOTS // Ka)
        r0 = 0
        while r0 < seg_rows:
            t = min(T, seg_rows - r0)
            chunks.append(("a", Ka, seg0 + r0, t))
            r0 += t
    for key in all_keys:
        Ki = key[1]
        if Ki == 0:
            continue
        nr = nrows[key]
        T = max(1, CHUNK_SLOTS // Ki)
        r0 = 0
        while r0 < nr:
            t = min(T, nr - r0)
            chunks.append(("i", Ki, row_off[key] + r0, t))
            r0 += t

    entries = []
    wins = []
    for table, K, g0, t in sorted(chunks, key=lambda c: -(c[3] * c[1])):
        w = t * K
        assert w % 2 == 0
        cw = w // 2 if fast_k else w
        for wi in range(len(wins)):
            if wins[wi] >= cw:
                break
        else:
            wins.append(V2_WINDOW)
            wi = len(wins) - 1
        woff = V2_WINDOW - wins[wi]
        entries.append((table, K, g0, t, wi, woff))
        wins[wi] -= cw
    n_windows = len(wins)
    win_used = [-(-(V2_WINDOW - rem) // 64) * 64 for rem in wins]
    win_start = [0]
    for u in win_used[:-1]:
        win_start.append(win_start[-1] + u)
    return entries, n_windows, win_used, win_start, row_off, act_seg_rows, total_rows


def _pack_v2(x, k_act, k_inh, nu, decay, growth, act_src, act_dst, inh_src, inh_dst,
             fast_k, fast_nodev, tables):
    a_src, a_k, a_deg, a_ptr, i_src, i_k, i_deg, i_ptr = tables

    max_deg = int(max(a_deg.max(), i_deg.max()))
    ks = _degree_classes(max_deg)
    nclasses = len(ks) + 1
    klist = [0] + ks

    ca = _class_of(a_deg, ks)
    ci = _class_of(i_deg, ks)

    kcap = min(16, ks[-1])
    pair_id = ca * 1024 + ci
    uniq_p, cnt_p = np.unique(pair_id, return_counts=True)
    rare_pairs = set(uniq_p[cnt_p < RARE_THRESH].tolist())
    if rare_pairs:
        rare = np.isin(pair_id, list(rare_pairs))
        ca = np.where(rare & (ca > 0), np.maximum(ca, kcap), ca)
        ci = np.where(rare & (ci > 0), np.maximum(ci, kcap), ci)

    core_bins = []
    for c in range(N_CORES):
        lo, hi = c * NPC, (c + 1) * NPC
        nodes = np.arange(lo, hi)
        binid = np.searchsorted(np.array(klist), ca[lo:hi]) * nclasses + np.searchsorted(
            np.array(klist), ci[lo:hi]
        )
        order = np.argsort(binid, kind="stable")
        nodes_sorted = nodes[order]
        binid_sorted = binid[order]
        uniq, starts = np.unique(binid_sorted, return_index=True)
        ends = np.append(starts[1:], len(binid_sorted))
        bins = {}
        for u, s, e in zip(uniq, starts, ends):
            bins[(klist[u // nclasses], klist[u % nclasses])] = nodes_sorted[s:e]
        core_bins.append(bins)

    all_keys = sorted({k for b in core_bins for k in b.keys()})
    nrows = {}
    for key in all_keys:
        nmax = max(len(b.get(key, ())) for b in core_bins)
        nrows[key] = -(-nmax // P)

    (entries, n_windows, win_used, win_start, row_off, act_seg_rows,
     total_rows) = _make_plan_v2(all_keys, nrows, fast_k)

    def build_slab(L, K, rowptr, deg, srcs, kvals, want_k):
        Lc = L.clip(0)
        d = np.where(L >= 0, deg[Lc], 0)
        base = rowptr[Lc]
        cols = np.arange(K)
        idx2 = base[:, None] + cols[None, :]
        valid = cols[None, :] < d[:, None]
        idxc = np.where(valid, idx2, 0)
        sx = np.where(valid, x[srcs[idxc]], np.float32(0)).astype(np.float32)
        sk = None
        if want_k:
            sk = np.where(valid, kvals[idxc], np.float32(0)).astype(np.float32)
        return sx, sk

    per_core = []
    meta_orders = []
    for c in range(N_CORES):
        bins = core_bins[c]
        ax_seg = {}
        ak_seg = {}
        ix_bin = {}
        ik_bin = {}
        xv_l = []
        nodev_l = {"nuv": [], "dev": [], "grv": []}
        orders = []
        ax_parts = {}
        ak_parts = {}
        for key in all_keys:
            Ka, Ki = key
            nr = nrows[key]
            L = np.full(nr * P, -1, np.int64)
            have = bins.get(key)
            if have is not None:
                L[: len(have)] = have
            orders.append((key, L))
            if Ka > 0:
                sx, sk = build_slab(L, Ka, a_ptr, a_deg, a_src, a_k, not fast_k)
                ax_parts.setdefault(Ka, []).append(sx.reshape(P, nr * Ka))
                if not fast_k:
                    ak_parts.setdefault(Ka, []).append(sk.reshape(P, nr * Ka))
            if Ki > 0:
                sx, sk = build_slab(L, Ki, i_ptr, i_deg, i_src, i_k, not fast_k)
                ix_bin[key] = sx.reshape(P, nr * Ki)
                if not fast_k:
                    ik_bin[key] = sk.reshape(P, nr * Ki)
            valid = L >= 0
            Lc = L.clip(0)

            def pk(v):
                return (
                    np.where(valid, v[Lc], np.float32(0))
                    .astype(np.float32)
                    .reshape(P, nr)
                )

            xv_l.append(pk(x))
            if not fast_nodev:
                nodev_l["nuv"].append(pk(nu))
                nodev_l["dev"].append(pk(decay))
                nodev_l["grv"].append(pk(growth))

        for Ka, parts in ax_parts.items():
            ax_seg[Ka] = np.concatenate(parts, axis=1)
            if not fast_k:
                ak_seg[Ka] = np.concatenate(ak_parts[Ka], axis=1)

        slab = np.zeros((P, win_start[-1] + win_used[-1]), np.float32)
        for table, K, g0, t, win, woff in entries:
            w = t * K
            base = win_start[win] + woff
            if table == "a":
                seg0, _ = act_seg_rows[K]
                r0 = g0 - seg0
                sx = ax_seg[K][:, r0 * K : (r0 + t) * K]
                sk = ak_seg[K][:, r0 * K : (r0 + t) * K] if not fast_k else None
            else:
                key = next(
                    kk for kk in all_keys
                    if kk[1] == K and row_off[kk] <= g0 < row_off[kk] + nrows[kk]
                )
                r0 = g0 - row_off[key]
                sx = ix_bin[key][:, r0 * K : (r0 + t) * K]
                sk = ik_bin[key][:, r0 * K : (r0 + t) * K] if not fast_k else None
            xw = w // 2
            slab[:, base : base + xw] = _pack_bf16_words(sx)
            if not fast_k:
                slab[:, base + xw : base + 2 * xw] = _pack_bf16_words(sk)

        core = {
            "slab": slab,
            "nodevf": np.ascontiguousarray(np.concatenate(xv_l, axis=1)),
        }
        if not fast_nodev:
            nodevb = np.concatenate(
                [np.concatenate(nodev_l[nm], axis=1) for nm in ("nuv", "dev", "grv")],
                axis=1,
            )
            core["nodevb"] = _pack_bf16_words(nodevb)
        per_core.append(core)
        meta_orders.append(orders)

    r00 = None
    if (0, 0) in nrows:
        r00 = (row_off[(0, 0)], nrows[(0, 0)])

    shapes = {
        "v3": False,
        "keys": all_keys,
        "nrows": nrows,
        "NR": total_rows,
        "entries": entries,
        "n_windows": n_windows,
        "win_used": win_used,
        "win_start": win_start,
        "fast_k": fast_k,
        "fast_nodev": fast_nodev,
        "r00": r00,
    }
    return per_core, meta_orders, shapes


def _build_nc_v2(shapes, loop_R=None):
    NR = shapes["NR"]
    entries = shapes["entries"]
    n_windows = shapes["n_windows"]
    win_used = shapes["win_used"]
    win_start = shapes["win_start"]
    fast_k = shapes["fast_k"]
    fast_nodev = shapes["fast_nodev"]
    r00 = shapes["r00"]

    NB = (3 * NR + 1) // 2
    nc = bacc.Bacc(None, target_bir_lowering=False)
    sl_d = nc.declare_dram_parameter(
        "slab", [P, win_start[-1] + win_used[-1]], F32, isOutput=False
    )
    nvf_d = nc.declare_dram_parameter("nodevf", [P, NR], F32, isOutput=False)
    nvb_d = None
    if not fast_nodev:
        nvb_d = nc.declare_dram_parameter("nodevb", [P, NB], F32, isOutput=False)
    out_d = nc.declare_dram_parameter("out", [P, NR], F32, isOutput=True)

    MUL = mybir.AluOpType.mult
    ADD = mybir.AluOpType.add
    X = mybir.AxisListType.X

    with tile.TileContext(nc) as tc:
        with (
            tc.tile_pool(name="slab", bufs=3) as slab_tp,
            tc.tile_pool(name="sums", bufs=1) as sums_tp,
            tc.tile_pool(name="node", bufs=1) as node_tp,
        ):
            loop_cm = tc.For_i(0, loop_R, 1) if loop_R else contextlib.nullcontext()
            with loop_cm:
                asum = sums_tp.tile([P, NR], F32, tag="asum")
                isum = sums_tp.tile([P, NR], F32, tag="isum")
                nc.vector.memset(asum[:, :], 1.0)
                nc.vector.memset(isum[:, :], 0.0)
                if r00 is not None:
                    nc.vector.memset(asum[:, r00[0] : r00[0] + r00[1]], 0.0)
                bufs = {"a": asum, "i": isum}

                nvf = node_tp.tile([P, NR], F32, tag="nvf")
                nc.scalar.dma_start(out=nvf[:, :], in_=nvf_d[:, :])
                if not fast_nodev:
                    nvb = node_tp.tile([P, NB], F32, tag="nvb")
                    nc.scalar.dma_start(out=nvb[:, :], in_=nvb_d[:, :])
                    nvb_b = nvb[:, :].bitcast(BF16)
                    iv = {}
                    for j, nm in enumerate(("nuv", "dev", "grv")):
                        iv[nm] = nvb_b[:, j * NR : (j + 1) * NR]

                by_win = {}
                for e in entries:
                    by_win.setdefault(e[4], []).append(e)
                sq_engine = 0
                for win in range(n_windows):
                    wt = slab_tp.tile([P, V2_WINDOW], F32, tag="win")
                    used = win_used[win]
                    half = (used // 2 + 63) // 64 * 64
                    half = min(half, used)
                    dma_eng = nc.sync
                    dma_eng.dma_start(
                        out=wt[:, :half],
                        in_=sl_d[:, win_start[win] : win_start[win] + half],
                    )
                    if half < used:
                        dma_eng.dma_start(
                            out=wt[:, half:used],
                            in_=sl_d[:, win_start[win] + half : win_start[win] + used],
                        )
                    for table, K, g0, t, _win, woff in by_win.get(win, ()):
                        w = t * K
                        xw = w // 2
                        xs = wt[:, woff : woff + xw].bitcast(BF16)
                        if fast_k:
                            if sq_engine == 0:
                                nc.scalar.square(out=xs, in_=xs)
                            else:
                                nc.gpsimd.tensor_tensor(out=xs, in0=xs, in1=xs, op=MUL)
                            sq_engine ^= 1
                        else:
                            kS = wt[:, woff + xw : woff + 2 * xw].bitcast(BF16)
                            nc.scalar.square(out=xs, in_=xs)
                            nc.gpsimd.tensor_tensor(out=xs, in0=xs, in1=kS, op=MUL)
                        nc.vector.tensor_reduce(
                            out=bufs[table][:, g0 : g0 + t],
                            in_=xs.rearrange("p (t k) -> p t k", k=K),
                            axis=X,
                            op=ADD,
                        )

                den = node_tp.tile([P, NR], F32, tag="den")
                rde = node_tp.tile([P, NR], F32, tag="rde")
                scr = node_tp.tile([P, NR], F32, tag="scr")
                ot = node_tp.tile([P, NR], F32, tag="ot")
                A = lambda tl: tl[:, :]

                nc.vector.tensor_scalar_add(A(den), A(isum), 1.0)
                nc.vector.reciprocal_approx_accurate(
                    out=A(rde), in_=A(den), scratch=A(scr)
                )
                nc.vector.tensor_tensor(out=A(rde), in0=A(asum), in1=A(rde), op=MUL)
                if fast_nodev:
                    nc.vector.scalar_tensor_tensor(
                        out=A(ot), in0=A(nvf), scalar=-1.0, in1=A(rde),
                        op0=MUL, op1=ADD,
                    )
                    nc.vector.tensor_scalar_add(A(ot), A(ot), 1.0)
                else:
                    nc.vector.tensor_tensor(out=A(ot), in0=iv["nuv"], in1=A(rde), op=MUL)
                    nc.vector.scalar_tensor_tensor(
                        out=A(scr), in0=iv["dev"], scalar=-1.0, in1=A(nvf),
                        op0=MUL, op1=MUL,
                    )
                    nc.vector.tensor_tensor(out=A(ot), in0=A(ot), in1=A(scr), op=ADD)
                    nc.vector.tensor_tensor(out=A(ot), in0=A(ot), in1=iv["grv"], op=ADD)
                nc.scalar.dma_start(out=out_d[:, :], in_=ot[:, :])

    nc.finalize()
    return nc


# ================= dispatchers =================
def _pack(x, k_act, k_inh, nu, decay, growth, act_src, act_dst, inh_src, inh_dst):
    fast_k = bool(np.all(k_act == 1.0) and np.all(k_inh == 1.0))
    fast_nodev = bool(
        np.all(nu == 1.0) and np.all(decay == 1.0) and np.all(growth == 1.0)
    )

    def sorted_table(src, dst, k):
        order = np.argsort(dst, kind="stable")
        deg = np.bincount(dst, minlength=N_NODES).astype(np.int64)
        rowptr = np.zeros(N_NODES + 1, np.int64)
        np.cumsum(deg, out=rowptr[1:])
        return src[order], k[order], deg, rowptr

    a_src, a_k, a_deg, a_ptr = sorted_table(act_src, act_dst, k_act)
    i_src, i_k, i_deg, i_ptr = sorted_table(inh_src, inh_dst, k_inh)

    if fast_k and fast_nodev:
        core_bins = _make_bins(a_deg, i_deg)
        return _pack_v3(
            x, None, a_src, a_deg, a_ptr, i_src, i_deg, i_ptr, core_bins
        )
    return _pack_v2(
        x, k_act, k_inh, nu, decay, growth, act_src, act_dst, inh_src, inh_dst,
        fast_k, fast_nodev,
        (a_src, a_k, a_deg, a_ptr, i_src, i_k, i_deg, i_ptr),
    )


def _build_nc(shapes, loop_R=None, variant="full"):
    if shapes.get("v3"):
        return _build_nc_v3(shapes, loop_R=loop_R, variant=variant)
    return _build_nc_v2(shapes, loop_R=loop_R)


def kernel(**inputs) -> np.ndarray:
    per_core, meta_orders, shapes = _pack(
        np.asarray(inputs["x"], np.float32),
        np.asarray(inputs["k_act"], np.float32),
        np.asarray(inputs["k_inh"], np.float32),
        np.asarray(inputs["nu"], np.float32),
        np.asarray(inputs["decay"], np.float32),
        np.asarray(inputs["growth"], np.float32),
        np.asarray(inputs["act_src"]),
        np.asarray(inputs["act_dst"]),
        np.asarray(inputs["inh_src"]),
        np.asarray(inputs["inh_dst"]),
    )
    nc = _build_nc(shapes)
    in_maps = [dict(per_core[c]) for c in range(N_CORES)]
    res = run_bass_kernel_spmd(nc, in_maps, list(range(N_CORES)))

    out_full = np.zeros(N_NODES, np.float32)
    nrows = shapes["nrows"]
    for c in range(N_CORES):
        arr = np.asarray(res.results[c]["out"]).astype(np.float32)
        offN = 0
        for key, L in meta_orders[c]:
            nr = nrows[key]
            block = arr[:, offN : offN + nr].reshape(P * nr)
            valid = L >= 0
            out_full[L[valid]] = block[valid]
            offN += nr
    return out_full
